# revision 1
# baseline (speedup 1.0000x reference)
"""Trainium2 Bass kernel for nn_DAttentionMM (deformable attention, multi-modal).

Strategy: data-parallel over batch B=8 across 8 NeuronCores. Each core runs the
full per-batch pipeline:
  conv3x3(+folded BN)+GELU -> q proj -> offset branch (dwconv/LN/GELU/pw) ->
  bilinear sampling of x, y, h -> sw mixing -> k/v proj -> 8-head attention
  (attnT layout, ones-augmented AV for softmax sums) -> output proj.

Host side folds BN into the conv weights, pre-transposes all 1x1-conv weights
into lhsT layout, pre-pads/transposes x,y into [5184, 256] gather tables, and
folds sw1@projq into a single M1 matrix so q never needs an on-device gather.
"""
import sys

sys.path.insert(0, '/opt/trn_rl_repo')

import numpy as np

B, C, H, W = 8, 256, 64, 64
NH, HC = 8, 32
Hk = Wk = 8
NS = 64
SCALE = float(HC) ** -0.5
EPS = 1e-5
HW = H * W
PADR = 72          # padded rows/cols for the stride-8 9x9 dwconv (+4 each side)
NROW = PADR * PADR  # 5184

_CACHE = {}
ATT_BF16 = False  # bf16 AV path: ~8% faster, adds ~2.3e-3 rel err


def _build_program():
    import concourse.bass as bass
    import concourse.tile as tile
    from concourse import bacc, mybir
    from concourse.masks import make_identity

    f32 = mybir.dt.float32
    f32r = mybir.dt.float32r
    i32 = mybir.dt.int32
    AF = mybir.ActivationFunctionType
    ALU = mybir.AluOpType
    ts = bass.ts

    nc = bacc.Bacc("TRN2", target_bir_lowering=False, debug=False)

    dp = lambda name, shape, dt=f32: nc.declare_dram_parameter(name, list(shape), dt, isOutput=False)
    xc = dp("xc", (C, H, W))
    yc = dp("yc", (C, H, W))
    xTp = dp("xTp", (NROW, C))
    yTp = dp("yTp", (NROW, C))
    fw = dp("fw", (4, 9, 128, 256))          # conv lhsT [ci, tap, p_in, m_out]
    fb = dp("fb", (128, 2))
    pqw = dp("pqw", (128, 4, 128))           # [p, ci*2+mo, m]
    pqb = dp("pqb", (128, 2))
    m1w = dp("m1w", (128, 4, 128))
    c1b = dp("c1b", (128, 2))
    sw2w = dp("sw2w", (128, 2, 2))           # [p, ci, t]
    sigb = dp("sigb", (128, 2))              # [+db, -db]
    pwx = dp("pwx", (128, 2, 2))             # [p, ci, t]
    pwy = dp("pwy", (128, 2, 2))
    dwsc = dp("dwsc", (128, 2, 2, 81))       # [p, ci, img, tap]
    dwbc = dp("dwbc", (128, 2, 2))           # [p, img, ci]
    lnGc = dp("lnGc", (128, 2, 2))
    lnBc = dp("lnBc", (128, 2, 2))
    ref2 = dp("ref2", (2, 64))
    pkw = dp("pkw", (128, 4, 128))
    pkb = dp("pkb", (128, 2))
    pvw = dp("pvw", (128, 4, 128))
    pvb = dp("pvb", (128, 2))
    pow_ = dp("pow", (128, 4, 128))
    pob = dp("pob", (128, 2))

    out_d = nc.declare_dram_parameter("out", [C, HW], f32, isOutput=True)
    hT_d = nc.dram_tensor("hT_scratch", [HW + 1, C], f32)
    posd = nc.dram_tensor("pos_scratch", [256], f32)

    with tile.TileContext(nc) as tc:
        import contextlib
        with contextlib.ExitStack() as ctx:
            const = ctx.enter_context(tc.tile_pool(name="const", bufs=1))
            work = ctx.enter_context(tc.tile_pool(name="work", bufs=1))

            # ---------- constant tiles (loads deferred; fw0+dwsc first) ----------
            fw_t = const.tile([128, 36, 256], f32r)
            fw_view = fw[:].rearrange("c t p m -> p (c t) m").bitcast(f32r)
            nc.gpsimd.dma_start(out=fw_t[:, 0:9, :], in_=fw_view[:, 0:9, :])
            dwsc_t = const.tile([128, 2, 2, 81], f32)
            nc.gpsimd.dma_start(out=dwsc_t, in_=dwsc[:])
            _dn = [0]
            _deferred = []
            def dtile(shape, dt, srcap):
                _dn[0] += 1
                t = const.tile(shape, dt, name=f"cw{_dn[0]}")
                _deferred.append((t, srcap))
                return t
            fb_t = dtile([128, 2], f32, fb[:])
            pqw_t = dtile([128, 4, 128], f32r, pqw[:].bitcast(f32r))
            pqb_t = dtile([128, 2], f32, pqb[:])
            m1w_t = dtile([128, 4, 128], f32r, m1w[:].bitcast(f32r))
            c1b_t = dtile([128, 2], f32, c1b[:])
            sw2w_t = dtile([128, 2, 2], f32r, sw2w[:].bitcast(f32r))
            sigb_t = dtile([128, 2], f32, sigb[:])
            pwx_t = dtile([128, 2, 2], f32, pwx[:])
            pwy_t = dtile([128, 2, 2], f32, pwy[:])
            dwbc_t = dtile([128, 2, 2], f32, dwbc[:])
            lnGc_t = dtile([128, 2, 2], f32, lnGc[:])
            lnBc_t = dtile([128, 2, 2], f32, lnBc[:])
            ref_t = dtile([2, 64], f32, ref2[:])
            pkw_t = dtile([128, 4, 128], f32r, pkw[:].bitcast(f32r))
            pkb_t = dtile([128, 2], f32, pkb[:])
            pvw_t = dtile([128, 4, 128], f32r, pvw[:].bitcast(f32r))
            pvb_t = dtile([128, 2], f32, pvb[:])
            pow_t = dtile([128, 4, 128], f32r, pow_[:].bitcast(f32r))
            pob_t = dtile([128, 2], f32, pob[:])
            ones_r = const.tile([128, 1], f32r)
            nc.vector.memset(ones_r.bitcast(f32), 1.0)
            att_dt = mybir.dt.bfloat16 if ATT_BF16 else f32r
            ones_m = const.tile([128, 32], att_dt)
            nc.vector.memset(ones_m if ATT_BF16 else ones_m.bitcast(f32), 1.0)
            ident = const.tile([128, 128], f32)
            make_identity(nc, ident)
            eps_t = const.tile([128, 1], f32)
            nc.vector.memset(eps_t, EPS)
            zrow = const.tile([1, 256], f32)
            nc.vector.memset(zrow, 0.0)
            nc.sync.dma_start(out=hT_d[HW:HW + 1, :], in_=zrow)

            # persistent activations
            q_t = work.tile([128, 2, HW], f32r)

            # =======================================================
            # Phase A: conv + offset branch + sampling prep
            # =======================================================
            with tc.tile_pool(name="convin", bufs=1) as cvp, \
                 tc.tile_pool(name="dwp", bufs=2) as dwp, \
                 tc.tile_pool(name="dwp1", bufs=1) as dwp1, \
                 tc.tile_pool(name="offp", bufs=1) as offp, \
                 tc.tile_pool(name="conv_ps", bufs=2, space="PSUM") as conv_ps, \
                 tc.tile_pool(name="tp_ps", bufs=2, space="PSUM") as tp_ps, \
                 tc.tile_pool(name="sm_ps", bufs=1, space="PSUM") as sm_ps:

                # ----- conv inputs, padded to [66, 66] -----
                pads = []
                pvs = []
                for cidx in range(4):
                    pt = cvp.tile([128, 72 * 72], f32r, name=f"pad{cidx}")
                    pv = pt[:, :].rearrange("p (r c) -> p r c", r=72)
                    pf = pt.bitcast(f32)[:, :].rearrange("p (r c) -> p r c", r=72)
                    nc.vector.memset(pf[:, 0:4, :], 0.0)
                    nc.vector.memset(pf[:, 68:72, :], 0.0)
                    nc.vector.memset(pf[:, 4:68, 0:4], 0.0)
                    nc.vector.memset(pf[:, 4:68, 68:72], 0.0)
                    pads.append(pt)
                    pvs.append(pv)
                for quarter in range(4):
                    r0, r1 = quarter * 16, quarter * 16 + 16
                    for cidx in range(4):
                        srcq = (xc if cidx < 2 else yc)[(cidx % 2) * 128:(cidx % 2) * 128 + 128]
                        eng = nc.sync if cidx % 2 == 0 else nc.scalar
                        eng.dma_start(out=pvs[cidx][:, 4 + r0:4 + r1, 4:68],
                                      in_=srcq[:, r0:r1, :].bitcast(f32r))
                    if quarter == 0:
                        for cs in range(1, 4):
                            nc.gpsimd.dma_start(out=fw_t[:, cs * 9:(cs + 1) * 9, :],
                                                in_=fw_view[:, cs * 9:(cs + 1) * 9, :])
                    if quarter == 2:
                        for _t, _srcap in _deferred:
                            nc.sync.dma_start(out=_t, in_=_srcap)


                # ----- dwconv (DVE, ch-part layout) reading the 72-padded conv tiles -----
                # dwacc[img][ci]: [128, 64]; acc in exact f32, f32r copy for stats
                hgc = {}
                for img in range(2):
                    accs = []
                    for ci in range(2):
                        pt = pads[img * 2 + ci]
                        acc576 = dwp.tile([128, 576], f32, tag="a576")
                        tmp576 = dwp.tile([128, 576], f32, tag="t576")
                        for ky in range(9):
                            sl = bass.AP(tensor=pt.tensor, offset=pt.offset + ky * 72,
                                         ap=[pt.ap[0], [576, 8], [8, 8], [1, 9]]).bitcast(f32)
                            wsl = dwsc_t[:, ci, img, ky * 9:(ky + 1) * 9]
                            wbc = bass.AP(tensor=wsl.tensor, offset=wsl.offset,
                                          ap=[wsl.ap[0], [0, 8], [0, 8], [1, 9]])
                            dst = acc576 if ky == 0 else tmp576
                            nc.vector.tensor_tensor(
                                out=dst[:, :].rearrange("p (a b c) -> p a b c", a=8, b=8),
                                in0=sl, in1=wbc, op=ALU.mult)
                            if ky > 0:
                                nc.vector.tensor_tensor(out=acc576, in0=acc576, in1=tmp576, op=ALU.add)
                        acc = offp.tile([128, 64], f32, name=f"dwacc{img}{ci}")
                        rview = bass.AP(tensor=acc576.tensor, offset=acc576.offset,
                                        ap=[acc576.ap[0], [9, 64], [1, 9]])
                        nc.vector.reduce_sum(out=acc, in_=rview, axis=mybir.AxisListType.X)
                        nc.vector.tensor_scalar(out=acc, in0=acc, scalar1=dwbc_t[:, img, ci:ci + 1],
                                                scalar2=None, op0=ALU.add)
                        accs.append(acc)
                    # LN stats over 256 channels (partitions, both chunks) via ones-matmul
                    accr = [offp.tile([128, 64], f32r, name=f"daccr{img}{ci}") for ci in range(2)]
                    sqr = [offp.tile([128, 64], f32r, name=f"dsqr{img}{ci}") for ci in range(2)]
                    for ci in range(2):
                        nc.vector.tensor_copy(accr[ci], accs[ci])
                        nc.vector.tensor_tensor(out=sqr[ci], in0=accs[ci], in1=accs[ci], op=ALU.mult)
                    ps_st = sm_ps.tile([1, 128], f32, tag="lnst")
                    for ci in range(2):
                        nc.tensor.matmul(ps_st[:, 0:64], ones_r, accr[ci], start=(ci == 0), stop=(ci == 1))
                    for ci in range(2):
                        nc.tensor.matmul(ps_st[:, 64:128], ones_r, sqr[ci], start=(ci == 0), stop=(ci == 1))
                    mean1 = offp.tile([1, 64], f32, name=f"m1_{img}")
                    nc.vector.tensor_scalar(out=mean1, in0=ps_st[:, 0:64], scalar1=1.0 / 256.0,
                                            scalar2=None, op0=ALU.mult)
                    ex2 = offp.tile([1, 64], f32, name=f"ex2_{img}")
                    nc.vector.tensor_scalar(out=ex2, in0=ps_st[:, 64:128], scalar1=1.0 / 256.0,
                                            scalar2=None, op0=ALU.mult)
                    msq = offp.tile([1, 64], f32, name=f"msq_{img}")
                    nc.vector.tensor_tensor(out=msq, in0=mean1, in1=mean1, op=ALU.mult)
                    var1 = offp.tile([1, 64], f32, name=f"var_{img}")
                    nc.vector.tensor_tensor(out=var1, in0=ex2, in1=msq, op=ALU.subtract)
                    std1 = offp.tile([1, 64], f32, name=f"std_{img}")
                    nc.scalar.activation(out=std1, in_=var1, func=AF.Sqrt, bias=eps_t[0:1, :], scale=1.0)
                    rstd1 = offp.tile([1, 64], f32, name=f"rstd_{img}")
                    nc.vector.reciprocal(out=rstd1, in_=std1)
                    mbc = offp.tile([128, 64], f32, name=f"mbc_{img}")
                    nc.gpsimd.partition_broadcast(mbc[:], mean1[0:1, :])
                    rbc = offp.tile([128, 64], f32, name=f"rbc_{img}")
                    nc.gpsimd.partition_broadcast(rbc[:], rstd1[0:1, :])
                    hgci = offp.tile([128, 2, 64], f32, name=f"hgc_{img}")
                    for ci in range(2):
                        t2 = dwp.tile([128, 64], f32, tag="dwtmp")
                        nc.vector.tensor_tensor(out=t2, in0=accs[ci], in1=mbc, op=ALU.subtract)
                        nc.vector.tensor_tensor(out=t2, in0=t2, in1=rbc, op=ALU.mult)
                        nc.vector.tensor_scalar(out=t2, in0=t2, scalar1=lnGc_t[:, img, ci:ci + 1],
                                                scalar2=None, op0=ALU.mult)
                        nc.vector.tensor_scalar(out=t2, in0=t2, scalar1=lnBc_t[:, img, ci:ci + 1],
                                                scalar2=None, op0=ALU.add)
                        nc.scalar.activation(out=hgci[:, ci, :], in_=t2, func=AF.Gelu, scale=1.0)
                    hgc[img] = hgci

                pos_sb = offp.tile([2, 2, 64], f32)   # [grid(x,y), (y..x..), 64]
                for g, pw_t in ((0, pwx_t), (1, pwy_t)):
                    pso = sm_ps.tile([2, 64], f32, tag="pso")
                    for ci in range(2):
                        nc.tensor.matmul(pso, pw_t[:, ci, :], hgc[g][:, ci, :],
                                         start=(ci == 0), stop=(ci == 1))
                    nc.vector.tensor_tensor(out=pos_sb[:, g, :], in0=pso, in1=ref_t, op=ALU.add)
                    nc.vector.tensor_scalar(out=pos_sb[:, g, :], in0=pos_sb[:, g, :],
                                            scalar1=-1.0, scalar2=1.0, op0=ALU.max, op1=ALU.min)
                    # interleave (y, x) pairs into DRAM: posd[g*128 + 2s + t]
                    nc.sync.dma_start(
                        out=bass.AP(tensor=posd, offset=g * 128, ap=[[1, 1], [1, 2], [2, 64]]),
                        in_=pos_sb[:, g, :])
                pos_pt = offp.tile([128, 2], f32)
                nc.sync.dma_start(out=pos_pt, in_=posd.ap().rearrange("(p t) -> p t", t=2))

                # ----- pixel coords, floor, weights, indices (all [128, *]) -----
                pix = offp.tile([128, 2], f32)
                nc.vector.tensor_scalar(out=pix, in0=pos_pt, scalar1=1.0, scalar2=31.5,
                                        op0=ALU.add, op1=ALU.mult)
                ri = offp.tile([128, 2], i32)
                nc.vector.tensor_copy(ri, pix)
                rf = offp.tile([128, 2], f32)
                nc.vector.tensor_copy(rf, ri)
                gt = offp.tile([128, 2], f32)
                nc.vector.tensor_tensor(out=gt, in0=rf, in1=pix, op=ALU.is_gt)
                base = offp.tile([128, 2], f32)
                nc.vector.tensor_tensor(out=base, in0=rf, in1=gt, op=ALU.subtract)
                wf = offp.tile([128, 2], f32)
                nc.vector.tensor_tensor(out=wf, in0=pix, in1=base, op=ALU.subtract)
                y1x1 = offp.tile([128, 2], f32)
                nc.vector.tensor_scalar(out=y1x1, in0=base, scalar1=1.0, scalar2=63.0,
                                        op0=ALU.add, op1=ALU.min)
                omw = offp.tile([128, 2], f32)   # 1 - w
                nc.vector.tensor_scalar(out=omw, in0=wf, scalar1=-1.0, scalar2=1.0,
                                        op0=ALU.mult, op1=ALU.add)
                wq = offp.tile([128, 4], f32)    # w00, w01, w10, w11
                nc.vector.tensor_tensor(out=wq[:, 0:1], in0=omw[:, 1:2], in1=omw[:, 0:1], op=ALU.mult)
                nc.vector.tensor_tensor(out=wq[:, 1:2], in0=wf[:, 1:2], in1=omw[:, 0:1], op=ALU.mult)
                nc.vector.tensor_tensor(out=wq[:, 2:3], in0=omw[:, 1:2], in1=wf[:, 0:1], op=ALU.mult)
                nc.vector.tensor_tensor(out=wq[:, 3:4], in0=wf[:, 1:2], in1=wf[:, 0:1], op=ALU.mult)
                # indices: cols 0=idxP(y0) 1=idxP(y1) 2=idx64(y0) 3=idx64(y1)
                idxf = offp.tile([128, 4], f32)
                nc.vector.tensor_scalar(out=idxf[:, 0:1], in0=base[:, 0:1], scalar1=72.0,
                                        scalar2=292.0, op0=ALU.mult, op1=ALU.add)
                nc.vector.tensor_tensor(out=idxf[:, 0:1], in0=idxf[:, 0:1], in1=base[:, 1:2], op=ALU.add)
                nc.vector.tensor_scalar(out=idxf[:, 1:2], in0=y1x1[:, 0:1], scalar1=72.0,
                                        scalar2=292.0, op0=ALU.mult, op1=ALU.add)
                nc.vector.tensor_tensor(out=idxf[:, 1:2], in0=idxf[:, 1:2], in1=base[:, 1:2], op=ALU.add)
                nc.vector.tensor_scalar(out=idxf[:, 2:3], in0=base[:, 0:1], scalar1=64.0,
                                        scalar2=None, op0=ALU.mult)
                nc.vector.tensor_tensor(out=idxf[:, 2:3], in0=idxf[:, 2:3], in1=base[:, 1:2], op=ALU.add)
                nc.vector.tensor_scalar(out=idxf[:, 3:4], in0=y1x1[:, 0:1], scalar1=64.0,
                                        scalar2=None, op0=ALU.mult)
                nc.vector.tensor_tensor(out=idxf[:, 3:4], in0=idxf[:, 3:4], in1=base[:, 1:2], op=ALU.add)
                idxi = offp.tile([128, 4], i32)
                nc.vector.tensor_copy(idxi, idxf)

                # ----- conv3x3 matmuls + gelu + fused projq + hT transposes -----
                for rb in range(8):
                    hb = dwp.tile([128, 2, 512], f32r, tag="hblk")
                    for mo in range(2):
                        ps = conv_ps.tile([128, 512], f32, tag="cps")
                        first = True
                        for ci in range(4):
                            pv = pads[ci][:, :].rearrange("p (r c) -> p r c", r=72)
                            for tap in range(9):
                                ky, kx = tap // 3, tap % 3
                                rhs = pv[:, rb * 8 + ky + 3: rb * 8 + ky + 11, kx + 3:kx + 67]
                                nc.tensor.matmul(ps, fw_t[:, ci * 9 + tap, ts(mo, 128)], rhs,
                                                 start=first, stop=(ci == 3 and tap == 8))
                                first = False
                        nc.scalar.activation(out=hb[:, mo, :], in_=ps,
                                             func=AF.Gelu, bias=fb_t[:, mo:mo + 1], scale=1.0)
                    for mo in range(2):
                        ps = conv_ps.tile([128, 512], f32, tag="cps")
                        for ci in range(2):
                            nc.tensor.matmul(ps, pqw_t[:, ci * 2 + mo, :], hb[:, ci, :],
                                             start=(ci == 0), stop=(ci == 1))
                        nc.scalar.activation(out=q_t[:, mo, ts(rb, 512)], in_=ps,
                                             func=AF.Identity, bias=pqb_t[:, mo:mo + 1], scale=1.0)
                    for ci in range(2):
                        for s4 in range(4):
                            tp = tp_ps.tile([128, 128], f32, tag="tp")
                            nc.tensor.transpose(tp, hb[:, ci, ts(s4, 128)].bitcast(f32), ident)
                            stg = dwp.tile([128, 128], f32, tag="hstage")
                            nc.scalar.activation(out=stg, in_=tp, func=AF.Copy, bias=0.0, scale=1.0)
                            nc.sync.dma_start(
                                out=hT_d[(rb * 4 + s4) * 128:(rb * 4 + s4 + 1) * 128, ts(ci, 128)],
                                in_=stg)

                # ----- strip gathers + bilinear -----
                def strip_gather(name, table, col):
                    g = dwp.tile([128, 512], f32, tag="strip")
                    nc.gpsimd.indirect_dma_start(
                        out=g[:], out_offset=None, in_=table,
                        in_offset=bass.IndirectOffsetOnAxis(ap=idxi[:, col:col + 1], axis=0))
                    return g

                def bilin(g0, g1, name):
                    o = offp.tile([128, 256], f32, name=name)
                    tmp = offp.tile([128, 256], f32, tag="btmp")
                    nc.vector.tensor_scalar(out=o, in0=g0[:, 0:256], scalar1=wq[:, 0:1], scalar2=None, op0=ALU.mult)
                    nc.vector.tensor_scalar(out=tmp, in0=g0[:, 256:512], scalar1=wq[:, 1:2], scalar2=None, op0=ALU.mult)
                    nc.vector.tensor_tensor(out=o, in0=o, in1=tmp, op=ALU.add)
                    nc.vector.tensor_scalar(out=tmp, in0=g1[:, 0:256], scalar1=wq[:, 2:3], scalar2=None, op0=ALU.mult)
                    nc.vector.tensor_tensor(out=o, in0=o, in1=tmp, op=ALU.add)
                    nc.vector.tensor_scalar(out=tmp, in0=g1[:, 256:512], scalar1=wq[:, 3:4], scalar2=None, op0=ALU.mult)
                    nc.vector.tensor_tensor(out=o, in0=o, in1=tmp, op=ALU.add)
                    return o

                xsT = bilin(strip_gather("xg0", xTp[:], 0), strip_gather("xg1", xTp[:], 1), "xsT")
                ysT = bilin(strip_gather("yg0", yTp[:], 0), strip_gather("yg1", yTp[:], 1), "ysT")
                hsT = bilin(strip_gather("hg0", hT_d[:].bitcast(f32), 2),
                            strip_gather("hg1", hT_d[:].bitcast(f32), 3), "hsT")

                # ----- sw branch: Z = relu(M1 @ hs + c1); S = sw2 @ Z; sw = sigmoid -----
                hs = offp.tile([128, 2, 128], f32r)
                for ci in range(2):
                    tp = tp_ps.tile([128, 128], f32, tag="tp")
                    nc.tensor.transpose(tp, hsT[:, ts(ci, 128)], ident)
                    nc.vector.tensor_copy(hs[:, ci, :], tp)
                zr = offp.tile([128, 2, 128], f32r)
                for mo in range(2):
                    ps = tp_ps.tile([128, 128], f32, tag="tp")
                    for ci in range(2):
                        nc.tensor.matmul(ps, m1w_t[:, ci * 2 + mo, :], hs[:, ci, :],
                                         start=(ci == 0), stop=(ci == 1))
                    nc.scalar.activation(out=zr[:, mo, :], in_=ps, func=AF.Relu,
                                         bias=c1b_t[:, mo:mo + 1], scale=1.0)
                psS = sm_ps.tile([2, 128], f32, tag="psS")
                for ci in range(2):
                    nc.tensor.matmul(psS, sw2w_t[:, ci, :], zr[:, ci, :],
                                     start=(ci == 0), stop=(ci == 1))
                S_sb = offp.tile([2, 128], f32)
                nc.vector.tensor_copy(S_sb, psS)
                psT = sm_ps.tile([128, 2], f32, tag="psT")
                nc.tensor.transpose(psT, S_sb, ident[0:2, 0:2])
                ST = offp.tile([128, 2], f32)
                nc.vector.tensor_copy(ST, psT)
                dS = offp.tile([128, 1], f32)
                nc.vector.tensor_tensor(out=dS, in0=ST[:, 0:1], in1=ST[:, 1:2], op=ALU.subtract)
                sw0 = offp.tile([128, 1], f32)
                nc.scalar.activation(out=sw0, in_=dS, func=AF.Sigmoid, bias=sigb_t[:, 0:1], scale=1.0)
                sw1 = offp.tile([128, 1], f32)
                nc.scalar.activation(out=sw1, in_=dS, func=AF.Sigmoid, bias=sigb_t[:, 1:2], scale=-1.0)

                # ----- sampled mix + transpose; k/v proj; vT_aug -----
                smT = offp.tile([128, 256], f32)
                tmp2 = offp.tile([128, 256], f32)
                nc.vector.tensor_scalar(out=smT, in0=xsT, scalar1=sw0, scalar2=None, op0=ALU.mult)
                nc.vector.tensor_scalar(out=tmp2, in0=ysT, scalar1=sw1, scalar2=None, op0=ALU.mult)
                nc.vector.tensor_tensor(out=smT, in0=smT, in1=tmp2, op=ALU.add)
                smpl = offp.tile([128, 2, 128], f32r)
                for ci in range(2):
                    tp = tp_ps.tile([128, 128], f32, tag="tp")
                    nc.tensor.transpose(tp, smT[:, ts(ci, 128)], ident)
                    nc.vector.tensor_copy(smpl[:, ci, :], tp)
                k_t = work.tile([128, 2, 128], f32r)
                v_t = work.tile([128, 2, 128], f32r)
                for dst, wt, bt in ((k_t, pkw_t, pkb_t), (v_t, pvw_t, pvb_t)):
                    for mo in range(2):
                        ps = tp_ps.tile([128, 128], f32, tag="tp")
                        for ci in range(2):
                            nc.tensor.matmul(ps, wt[:, ci * 2 + mo, :], smpl[:, ci, :],
                                             start=(ci == 0), stop=(ci == 1))
                        nc.scalar.activation(out=dst[:, mo, :], in_=ps, func=AF.Identity,
                                             bias=bt[:, mo:mo + 1], scale=1.0)
                vT8 = work.tile([128, 8, 32], att_dt)
                for ci in range(2):
                    tp = tp_ps.tile([128, 128], f32, tag="tp")
                    nc.tensor.transpose(tp, v_t[:, ci, :].bitcast(f32), ident)
                    for j in range(4):
                        nc.vector.tensor_copy(vT8[:, ci * 4 + j, :], tp[:, ts(j, 32)])

            # =======================================================
            # Phase B: attention + output projection
            # =======================================================
            with tc.tile_pool(name="apool", bufs=1) as apool, \
                 tc.tile_pool(name="epool", bufs=3) as epool, \
                 tc.tile_pool(name="npool", bufs=4) as npool, \
                 tc.tile_pool(name="opool", bufs=3) as opool, \
                 tc.tile_pool(name="qk_ps", bufs=2, space="PSUM") as qk_ps, \
                 tc.tile_pool(name="av_ps", bufs=2, space="PSUM") as av_ps, \
                 tc.tile_pool(name="po_ps", bufs=2, space="PSUM") as po_ps:

                att_t = apool.tile([128, 2, HW], f32r)
                E_tiles = {}

                def stage_qk(nb):
                    E = epool.tile([128, 8, 512], att_dt, tag="E")
                    for hg4 in range(2):
                        qks = []
                        for j in range(4):
                            qk = qk_ps.tile([128, 512], f32, tag="qk")
                            nc.tensor.matmul(qk, k_t[ts(j, 32), hg4, :],
                                             q_t[ts(j, 32), hg4, ts(nb, 512)],
                                             start=True, stop=True,
                                             tile_position=(32 * j, 0))
                            qks.append(qk)
                        for j in range(4):
                            nc.scalar.activation(out=E[:, hg4 * 4 + j, :], in_=qks[j],
                                                 func=AF.Exp, scale=SCALE)
                    E_tiles[nb] = E

                def stage_av(nb):
                    E = E_tiles.pop(nb)
                    if not ATT_BF16:
                        for hh in range(8):
                            av = av_ps.tile([32, 512], f32, tag="avg")
                            nc.tensor.matmul(av, vT8[:, hh, :], E[:, hh, :],
                                             start=True, stop=True)
                            sm = av_ps.tile([32, 512], f32, tag="sums")
                            nc.tensor.matmul(sm, ones_m, E[:, hh, :],
                                             start=True, stop=True)
                            rec = npool.tile([32, 512], f32, tag="rec")
                            nc.vector.reciprocal(out=rec, in_=sm)
                            nc.vector.tensor_tensor(out=att_t[ts(hh % 4, 32), hh // 4, ts(nb, 512)],
                                                    in0=av, in1=rec, op=ALU.mult)
                        return
                    for g in range(2):
                        avg = av_ps.tile([128, 512], f32, tag="avg")
                        ps_s = av_ps.tile([128, 512], f32, tag="sums")
                        for j in range(4):
                            hh = g * 4 + j
                            nc.tensor.matmul(avg[ts(j, 32), :], vT8[:, hh, :], E[:, hh, :],
                                             start=True, stop=True, tile_position=(0, 32 * j))
                            nc.tensor.matmul(ps_s[ts(j, 32), :], ones_m, E[:, hh, :],
                                             start=True, stop=True, tile_position=(0, 32 * j))
                        rec = npool.tile([128, 512], f32, tag="rec")
                        nc.vector.reciprocal(out=rec, in_=ps_s)
                        nc.vector.tensor_tensor(out=att_t[:, g, ts(nb, 512)],
                                                in0=avg, in1=rec, op=ALU.mult)

                def stage_po(nb):
                    for mo in range(2):
                        ps = po_ps.tile([128, 512], f32, tag="po")
                        for ci in range(2):
                            nc.tensor.matmul(ps, pow_t[:, ci * 2 + mo, :], att_t[:, ci, ts(nb, 512)],
                                             start=(ci == 0), stop=(ci == 1))
                        ot = opool.tile([128, 512], f32, tag="ot")
                        nc.scalar.activation(out=ot, in_=ps, func=AF.Identity,
                                             bias=pob_t[:, mo:mo + 1], scale=1.0)
                        nc.sync.dma_start(out=out_d[ts(mo, 128), ts(nb, 512)], in_=ot)

                for step in range(10):
                    if step < 8:
                        stage_qk(step)
                    if 1 <= step <= 8:
                        stage_av(step - 1)
                    if step >= 2:
                        stage_po(step - 2)

    nc.finalize()
    return nc


def _host_prep(inp):
    g = {k: np.ascontiguousarray(np.asarray(v, dtype=np.float32)) for k, v in inp.items()}
    s = g['bn_g'] / np.sqrt(g['bn_v'] + EPS)
    fwf = g['fuse_w'] * s[:, None, None, None]          # [256, 512, 3, 3]
    fbf = (g['fuse_b'] - g['bn_m']) * s + g['bn_b']
    M1 = g['sw1_w'] @ g['projq_w']
    c1 = g['sw1_w'] @ g['projq_b'] + g['sw1_b']

    def lhsT4(wmat):  # [out, in] -> [128, 4(ci*2+mo), 128]
        a = np.zeros((128, 4, 128), np.float32)
        for ci in range(2):
            for mo in range(2):
                a[:, ci * 2 + mo, :] = wmat[mo * 128:(mo + 1) * 128, ci * 128:(ci + 1) * 128].T
        return a

    def b2(vec):  # [256] -> [128, 2]
        return np.stack([vec[0:128], vec[128:256]], 1).astype(np.float32)

    d = {}
    fw_a = np.zeros((4, 9, 128, 256), np.float32)
    for ci in range(4):
        for ky in range(3):
            for kx in range(3):
                fw_a[ci, ky * 3 + kx] = fwf[:, ci * 128:(ci + 1) * 128, ky, kx].T
    d['fw'] = fw_a
    d['fb'] = b2(fbf)
    d['pqw'] = lhsT4(g['projq_w']); d['pqb'] = b2(g['projq_b'])
    d['m1w'] = lhsT4(M1);           d['c1b'] = b2(c1)
    sw2a = np.zeros((128, 2, 2), np.float32)
    for ci in range(2):
        sw2a[:, ci, :] = g['sw2_w'][:, ci * 128:(ci + 1) * 128].T
    d['sw2w'] = sw2a
    db = float(g['sw2_b'][0] - g['sw2_b'][1])
    d['sigb'] = np.tile(np.array([[db, -db]], np.float32), (128, 1))
    for nm, key in (('pwx', 'offx_pw_w'), ('pwy', 'offy_pw_w')):
        a = np.zeros((128, 2, 2), np.float32)
        for ci in range(2):
            a[:, ci, :] = g[key][:, ci * 128:(ci + 1) * 128].T
        d[nm] = a
    dwsc = np.zeros((128, 2, 2, 81), np.float32)
    dwbc = np.zeros((128, 2, 2), np.float32)
    lnGc = np.zeros((128, 2, 2), np.float32)
    lnBc = np.zeros((128, 2, 2), np.float32)
    for img, pre in ((0, 'offx'), (1, 'offy')):
        w = g[pre + '_dw_w'][:, 0].reshape(256, 81)
        for ci in range(2):
            dwsc[:, ci, img, :] = w[ci * 128:(ci + 1) * 128]
            dwbc[:, img, ci] = g[pre + '_dw_b'][ci * 128:(ci + 1) * 128]
            lnGc[:, img, ci] = g[pre + '_ln_g'][ci * 128:(ci + 1) * 128]
            lnBc[:, img, ci] = g[pre + '_ln_b'][ci * 128:(ci + 1) * 128]
    d['dwsc'] = dwsc; d['dwbc'] = dwbc; d['lnGc'] = lnGc; d['lnBc'] = lnBc
    ry = (np.linspace(0.5, Hk - 0.5, Hk, dtype=np.float32) / np.float32(Hk - 1.0)) * 2.0 - 1.0
    gy, gx = np.meshgrid(ry, ry, indexing='ij')
    d['ref2'] = np.stack([gy, gx], 0).reshape(2, 64).astype(np.float32)
    d['pkw'] = lhsT4(g['projk_w']); d['pkb'] = b2(g['projk_b'])
    d['pvw'] = lhsT4(g['projv_w']); d['pvb'] = b2(g['projv_b'])
    d['pow'] = lhsT4(g['projo_w']); d['pob'] = b2(g['projo_b'])
    return g, d


def kernel(**inputs):
    from concourse.bass_utils import run_bass_kernel_spmd

    if 'nc' not in _CACHE:
        _CACHE['nc'] = _build_program()
    nc = _CACHE['nc']

    g, wd = _host_prep(inputs)
    in_maps = []
    for b in range(B):
        m = dict(wd)
        xb = g['x'][b]
        yb = g['y'][b]
        m['xc'] = xb
        m['yc'] = yb
        for nm, img in (('xTp', xb), ('yTp', yb)):
            t = np.zeros((PADR, PADR, C), np.float32)
            t[4:68, 4:68] = img.transpose(1, 2, 0)
            m[nm] = t.reshape(NROW, C)
        in_maps.append(m)

    res = run_bass_kernel_spmd(nc, in_maps, list(range(B)))
    out = np.stack([res.results[i]['out'].reshape(C, H, W) for i in range(B)])
    return out.astype(np.float32)



# revision 12
# speedup vs baseline: 1.8481x; 1.8481x over previous
"""Trainium2 Bass kernel for nn_DAttentionMM (deformable attention, multi-modal).

Strategy: data-parallel over batch B=8 across 8 NeuronCores. Each core runs the
full per-batch pipeline:
  conv3x3 (fp8e4m3 DoubleRow matmuls, 2x256-row contraction per instr) + GELU
  -> q proj (bf16) -> offset branch (fp16 dwconv on DVE, LN via ones-matmul
  stats, erf-gelu) -> bilinear sampling of x, y (bf16 DRAM tables) and h (bf16
  hT scratch) -> sw mixing -> k/v proj -> 8-head attention in bf16 (packed
  4-head AV psum quadrants, ones-matmul softmax sums) -> output proj with the
  bias folded in as a ones-row matmul, stored straight from PSUM.

Precision plan (validated host-side against the jax reference):
  conv inputs+weights fp8e4m3 (weights pre-scaled x64, rescaled in the GELU),
  offset branch fp16, sampling tables / q / k / v / E bf16, everything else
  f32.  Expected rel err ~7e-3 vs the 2e-2 gate.
"""
import sys

sys.path.insert(0, '/opt/trn_rl_repo')

import numpy as np

B, C, H, W = 8, 256, 64, 64
NH, HC = 8, 32
Hk = Wk = 8
NS = 64
SCALE = float(HC) ** -0.5
EPS = 1e-5
HW = H * W
PADR = 72          # padded rows/cols for the stride-8 9x9 dwconv (+4 each side)
NROW = PADR * PADR  # 5184
WS = 64.0          # fp8 conv weight pre-scale (undone in the gelu activation)

CONV_FP8 = True    # fp8 DoubleRow conv path; False = bf16 9-tap fallback

_CACHE = {}


def _build_program():
    import concourse.bass as bass
    import concourse.tile as tile
    from concourse import bacc, mybir
    from concourse.masks import make_identity

    f32 = mybir.dt.float32
    i32 = mybir.dt.int32
    bf16 = mybir.dt.bfloat16
    f16 = mybir.dt.float16
    f8 = mybir.dt.float8e4
    u8 = mybir.dt.uint8
    AF = mybir.ActivationFunctionType
    ALU = mybir.AluOpType
    PM = mybir.MatmulPerfMode
    ts = bass.ts

    nc = bacc.Bacc("TRN2", target_bir_lowering=False, debug=False)

    dp = lambda name, shape, dt=f32: nc.declare_dram_parameter(name, list(shape), dt, isOutput=False)
    # conv inputs: 3 horizontal shifts x 2 imgs x 2 ci chunks, fp8 (as uint8)
    xs8 = dp("xs8", (128, 3, 2, 2, 4096), u8)
    fw8 = dp("fw8", (128, 9, 2, 2, 2, 128), u8)   # [p, tap, img, mo, i, m]
    xs16 = None
    fw16 = None
    if not CONV_FP8:
        xs16 = dp("xs16", (128, 3, 2, 2, 4096), bf16)
        fw16 = dp("fw16", (128, 9, 4, 2, 128), bf16)  # [p, tap, ci, mo, m]
    dwp16 = dp("dwp16", (128, 2, 2, NROW), f16)   # [p, img, ci, 72*72]
    fb = dp("fb", (128, 2))
    pqw = dp("pqw", (128, 4, 128), bf16)          # [p, ci*2+mo, m]
    pqb = dp("pqb", (128, 2))
    m1w = dp("m1w", (128, 4, 128), bf16)
    c1b = dp("c1b", (128, 2))
    sw2w = dp("sw2w", (128, 2, 2), bf16)          # [p, ci, t]
    sigb = dp("sigb", (128, 2))                   # [+db, -db]
    pwx = dp("pwx", (128, 2, 2))                  # [p, ci, t] (x0.5 folded)
    pwy = dp("pwy", (128, 2, 2))
    dwsc = dp("dwsc", (128, 2, 2, 81), f16)       # [p, ci, img, tap]
    dwbc = dp("dwbc", (128, 2, 2))                # [p, img, ci]
    lnGc = dp("lnGc", (128, 2, 2))
    lnBc = dp("lnBc", (128, 2, 2))
    ref2 = dp("ref2", (2, 64))
    pkw = dp("pkw", (128, 4, 128), bf16)
    pkb = dp("pkb", (128, 2))
    pvw = dp("pvw", (128, 4, 128), bf16)
    pvb = dp("pvb", (128, 2))
    pow_ = dp("pow", (128, 4, 128), bf16)
    pobr = dp("pobr", (1, 2, 128), bf16)          # out-proj bias rows
    xT16 = dp("xT16", (NROW, 256), bf16)
    yT16 = dp("yT16", (NROW, 256), bf16)

    out_d = nc.declare_dram_parameter("out", [C, HW], f32, isOutput=True)
    hT_d = nc.dram_tensor("hT_scratch", [HW + 1, 256], bf16)

    with tile.TileContext(nc) as tc:
        import contextlib
        with contextlib.ExitStack() as ctx:
            const = ctx.enter_context(tc.tile_pool(name="const", bufs=1))
            work = ctx.enter_context(tc.tile_pool(name="work", bufs=1))

            # ---------------- constants / weight tiles ----------------
            ident16 = const.tile([128, 128], bf16)
            make_identity(nc, ident16)
            identf = const.tile([2, 2], f32)
            make_identity(nc, identf)
            ones_bf = const.tile([1, 512], bf16)
            nc.vector.memset(ones_bf, 1.0)
            ones_m = const.tile([128, 32], bf16)
            nc.vector.memset(ones_m, 1.0)
            onesr = const.tile([128, 1], f32)
            nc.vector.memset(onesr, 1.0)
            zrow = const.tile([1, 256], bf16)
            nc.vector.memset(zrow, 0.0)
            nc.sync.dma_start(out=hT_d[HW:HW + 1, :], in_=zrow)

            # big input tiles
            if CONV_FP8:
                xs_t = const.tile([128, 3, 2, 2, 4096], f8)
                fw_t = const.tile([128, 9, 2, 2, 2, 128], f8)
                nc.gpsimd.dma_start(out=fw_t, in_=fw8[:].bitcast(f8))
                nc.sync.dma_start(out=xs_t[:, 0], in_=xs8[:, 0].bitcast(f8))
                nc.scalar.dma_start(out=xs_t[:, 1], in_=xs8[:, 1].bitcast(f8))
                nc.gpsimd.dma_start(out=xs_t[:, 2], in_=xs8[:, 2].bitcast(f8))
            else:
                xs_t = const.tile([128, 3, 2, 2, 4096], bf16)
                fw_t = const.tile([128, 9, 4, 2, 128], bf16)
                nc.gpsimd.dma_start(out=fw_t, in_=fw16[:])
                nc.sync.dma_start(out=xs_t[:, 0], in_=xs16[:, 0])
                nc.scalar.dma_start(out=xs_t[:, 1], in_=xs16[:, 1])
                nc.gpsimd.dma_start(out=xs_t[:, 2], in_=xs16[:, 2])
            dwp_t = const.tile([128, 2, 2, NROW], f16)
            nc.sync.dma_start(out=dwp_t[:, 0], in_=dwp16[:, 0])
            nc.scalar.dma_start(out=dwp_t[:, 1], in_=dwp16[:, 1])

            # small weights (one batched queue on gpsimd after fw/xs)
            _dn = [0]
            def ctile(shape, dt, srcap):
                _dn[0] += 1
                t = const.tile(shape, dt, name=f"cw{_dn[0]}")
                nc.gpsimd.dma_start(out=t, in_=srcap)
                return t
            dwsc_t = ctile([128, 2, 2, 81], f16, dwsc[:])
            dwbc_t = ctile([128, 2, 2], f32, dwbc[:])
            lnGc_t = ctile([128, 2, 2], f32, lnGc[:])
            lnBc_t = ctile([128, 2, 2], f32, lnBc[:])
            ref_t = ctile([2, 64], f32, ref2[:])
            pwx_t = ctile([128, 2, 2], f32, pwx[:])
            pwy_t = ctile([128, 2, 2], f32, pwy[:])
            fb_t = ctile([128, 2], f32, fb[:])
            pqw_t = ctile([128, 4, 128], bf16, pqw[:])
            pqb_t = ctile([128, 2], f32, pqb[:])
            m1w_t = ctile([128, 4, 128], bf16, m1w[:])
            c1b_t = ctile([128, 2], f32, c1b[:])
            sw2w_t = ctile([128, 2, 2], bf16, sw2w[:])
            sigb_t = ctile([128, 2], f32, sigb[:])
            pkw_t = ctile([128, 4, 128], bf16, pkw[:])
            pkb_t = ctile([128, 2], f32, pkb[:])
            pvw_t = ctile([128, 4, 128], bf16, pvw[:])
            pvb_t = ctile([128, 2], f32, pvb[:])
            pow_t = ctile([128, 4, 128], bf16, pow_[:])
            pobr_t = ctile([1, 2, 128], bf16, pobr[:])

            # persistent activations
            q_t = work.tile([128, 2, HW], bf16)
            k_t = work.tile([128, 2, 128], bf16)
            vT8 = work.tile([128, 8, 32], bf16)

            # =======================================================
            # Phase A: offset branch + conv + sampling prep
            # =======================================================
            with tc.tile_pool(name="dwp", bufs=2) as dwp, \
                 tc.tile_pool(name="offp", bufs=1) as offp, \
                 tc.tile_pool(name="hbp", bufs=2) as hbp, \
                 tc.tile_pool(name="stp", bufs=2) as stp, \
                 tc.tile_pool(name="gthp", bufs=2) as gthp, \
                 tc.tile_pool(name="conv_ps", bufs=2, space="PSUM") as conv_ps, \
                 tc.tile_pool(name="pq_ps", bufs=2, space="PSUM") as pq_ps, \
                 tc.tile_pool(name="tp_ps", bufs=2, space="PSUM") as tp_ps, \
                 tc.tile_pool(name="sm_ps", bufs=2, space="PSUM") as sm_ps:

                _spn = [0]
                def small_ps():
                    # rotating [128, 128] f32 bank for all small psum uses
                    _spn[0] += 1
                    return sm_ps.tile([128, 128], f32, tag="smallf", name=f"sp{_spn[0]}")

                # ----- dwconv on DVE (fp16), issued first so DVE runs during conv -----
                accs = {}
                for img in range(2):
                    for ci in range(2):
                        acc576 = dwp.tile([128, 576], f16, tag="a576")
                        tmp576 = dwp.tile([128, 576], f16, tag="t576")
                        base = dwp_t.offset + (img * 2 + ci) * NROW
                        for ky in range(9):
                            sl = bass.AP(tensor=dwp_t.tensor, offset=base + ky * 72,
                                         ap=[dwp_t.ap[0], [576, 8], [8, 8], [1, 9]])
                            wsl = dwsc_t[:, ci, img, ky * 9:(ky + 1) * 9]
                            wbc = bass.AP(tensor=wsl.tensor, offset=wsl.offset,
                                          ap=[wsl.ap[0], [0, 8], [0, 8], [1, 9]])
                            dst = acc576 if ky == 0 else tmp576
                            nc.vector.tensor_tensor(
                                out=dst[:, :].rearrange("p (a b c) -> p a b c", a=8, b=8),
                                in0=sl, in1=wbc, op=ALU.mult)
                            if ky > 0:
                                nc.vector.tensor_tensor(out=acc576, in0=acc576, in1=tmp576, op=ALU.add)
                        acc = offp.tile([128, 64], f32, name=f"dwacc{img}{ci}")
                        rview = bass.AP(tensor=acc576.tensor, offset=acc576.offset,
                                        ap=[acc576.ap[0], [9, 64], [1, 9]])
                        nc.vector.reduce_sum(out=acc, in_=rview, axis=mybir.AxisListType.X)
                        nc.vector.tensor_scalar(out=acc, in0=acc, scalar1=dwbc_t[:, img, ci:ci + 1],
                                                scalar2=None, op0=ALU.add)
                        accs[(img, ci)] = acc

                # ----- conv3x3 + gelu + projq + hT transposes, software-pipelined -----
                def conv_block(rb):
                    hb = hbp.tile([128, 2, 512], bf16, tag="hblk")
                    for mo in range(2):
                        ps = conv_ps.tile([128, 512], f32, tag="cps")
                        first = True
                        nmm = 18 if CONV_FP8 else 36
                        done = 0
                        for dy in (0, -1, 1):
                            for dx in (0, -1, 1):
                                tap = (dy + 1) * 3 + (dx + 1)
                                s = dx + 1
                                o0, o1 = 0, 512
                                if rb == 0 and dy == -1:
                                    o0 = 64
                                if rb == 7 and dy == 1:
                                    o1 = 448
                                r0 = rb * 512 + dy * 64 + o0
                                n = o1 - o0
                                if CONV_FP8:
                                    for img in range(2):
                                        done += 1
                                        nc.tensor.matmul(
                                            ps[:, o0:o1], fw_t[:, tap, img, mo],
                                            xs_t[:, s, img, :, r0:r0 + n],
                                            start=first, stop=(done == nmm),
                                            perf_mode=PM.DoubleRow,
                                            skip_group_check=True)
                                        first = False
                                else:
                                    for cidx in range(4):
                                        done += 1
                                        nc.tensor.matmul(
                                            ps[:, o0:o1], fw_t[:, tap, cidx, mo],
                                            xs_t[:, s, cidx // 2, cidx % 2, r0:r0 + n],
                                            start=first, stop=(done == nmm),
                                            skip_group_check=True)
                                        first = False
                        nc.scalar.activation(out=hb[:, mo, :], in_=ps, func=AF.Gelu,
                                             bias=fb_t[:, mo:mo + 1],
                                             scale=(1.0 / WS) if CONV_FP8 else 1.0)
                    return hb

                def q_and_transpose(rb, hb):
                    for mo in range(2):
                        ps = pq_ps.tile([128, 512], f32, tag="pqs")
                        for ci in range(2):
                            nc.tensor.matmul(ps, pqw_t[:, ci * 2 + mo, :], hb[:, ci, :],
                                             start=(ci == 0), stop=(ci == 1))
                        nc.scalar.activation(out=q_t[:, mo, ts(rb, 512)], in_=ps,
                                             func=AF.Identity, bias=pqb_t[:, mo:mo + 1], scale=1.0)
                    stg = stp.tile([128, 4, 2, 128], bf16, tag="hstage")
                    for ci in range(2):
                        tp = tp_ps.tile([128, 512], bf16, tag="tp4")
                        for s4 in range(4):
                            nc.tensor.transpose(tp[:, ts(s4, 128)], hb[:, ci, ts(s4, 128)],
                                                ident16)
                        nc.scalar.activation(
                            out=bass.AP(tensor=stg.tensor, offset=stg.offset + ci * 128,
                                        ap=[stg.ap[0], [256, 4], [1, 128]]),
                            in_=tp, func=AF.Copy, bias=0.0, scale=1.0)
                    nc.sync.dma_start(out=hT_d[rb * 512:(rb + 1) * 512, :]
                                      .rearrange("(s p) c -> p s c", p=128),
                                      in_=stg)

                hb_prev = None
                for rb in range(8):
                    hb = conv_block(rb)
                    if hb_prev is not None:
                        q_and_transpose(rb - 1, hb_prev)
                    hb_prev = hb
                q_and_transpose(7, hb_prev)

                # ----- LN over 256 channels via ones-matmul stats -----
                var2 = offp.tile([1, 2, 64], f32)
                mean_ = offp.tile([1, 2, 64], f32)
                for img in range(2):
                    sqs = []
                    for ci in range(2):
                        sq = dwp.tile([128, 64], f32, tag="dwsq")
                        nc.vector.tensor_tensor(out=sq, in0=accs[(img, ci)],
                                                in1=accs[(img, ci)], op=ALU.mult)
                        sqs.append(sq)
                    ps_s1 = small_ps()
                    for ci in range(2):
                        nc.tensor.matmul(ps_s1[0:1, 0:64], onesr, accs[(img, ci)],
                                         start=(ci == 0), stop=(ci == 1))
                    ps_s2 = small_ps()
                    for ci in range(2):
                        nc.tensor.matmul(ps_s2[0:1, 0:64], onesr, sqs[ci],
                                         start=(ci == 0), stop=(ci == 1))
                    nc.vector.tensor_scalar(out=mean_[:, img, :], in0=ps_s1[0:1, 0:64],
                                            scalar1=1.0 / 256.0, scalar2=None, op0=ALU.mult)
                    ex2 = offp.tile([1, 64], f32, tag="ex2")
                    nc.vector.tensor_scalar(out=ex2, in0=ps_s2[0:1, 0:64],
                                            scalar1=1.0 / 256.0, scalar2=None, op0=ALU.mult)
                    msq = offp.tile([1, 64], f32, tag="msq")
                    nc.vector.tensor_tensor(out=msq, in0=mean_[:, img, :], in1=mean_[:, img, :], op=ALU.mult)
                    nc.vector.tensor_tensor(out=var2[:, img, :], in0=ex2, in1=msq, op=ALU.subtract)
                    nc.vector.tensor_scalar(out=var2[:, img, :], in0=var2[:, img, :],
                                            scalar1=EPS, scalar2=None, op0=ALU.add)
                # one table switch: rstd = sqrt(1/var) for both imgs in one Sqrt
                rvar = offp.tile([1, 2, 64], f32)
                nc.vector.reciprocal(out=rvar.rearrange("a b c -> a (b c)"),
                                     in_=var2.rearrange("a b c -> a (b c)"))
                rstd2 = offp.tile([1, 2, 64], f32)
                nc.scalar.activation(out=rstd2.rearrange("a b c -> a (b c)"),
                                     in_=rvar.rearrange("a b c -> a (b c)"),
                                     func=AF.Sqrt, bias=0.0, scale=1.0)

                pos_sb = offp.tile([2, 2, 64], f32)   # [yx, grid, s]
                for img in range(2):
                    mbc = offp.tile([128, 64], f32, tag="mbc")
                    nc.gpsimd.partition_broadcast(mbc[:], mean_[0:1, img, :])
                    rbc = offp.tile([128, 64], f32, tag="rbc")
                    nc.gpsimd.partition_broadcast(rbc[:], rstd2[0:1, img, :])
                    hgci = offp.tile([128, 2, 64], f32, name=f"hgc_{img}")
                    for ci in range(2):
                        t2 = dwp.tile([128, 64], f32, tag="dwt2")
                        nc.vector.tensor_tensor(out=t2, in0=accs[(img, ci)], in1=mbc, op=ALU.subtract)
                        nc.vector.tensor_tensor(out=t2, in0=t2, in1=rbc, op=ALU.mult)
                        nc.vector.tensor_scalar(out=t2, in0=t2, scalar1=lnGc_t[:, img, ci:ci + 1],
                                                scalar2=None, op0=ALU.mult)
                        nc.vector.tensor_scalar(out=hgci[:, ci, :], in0=t2,
                                                scalar1=lnBc_t[:, img, ci:ci + 1],
                                                scalar2=None, op0=ALU.add)
                    # exact gelu via erf (sigmoid table): g = x + x*erf(x/sqrt2); 0.5 in pw
                    ebuf = dwp.tile([128, 2, 64], f32, tag="erf")
                    nc.scalar.activation(out=ebuf.rearrange("p a b -> p (a b)"),
                                         in_=hgci.rearrange("p a b -> p (a b)"),
                                         func=AF.Erf, bias=0.0, scale=float(2.0 ** -0.5))
                    nc.vector.tensor_tensor(out=ebuf, in0=ebuf, in1=hgci, op=ALU.mult)
                    nc.vector.tensor_tensor(out=hgci, in0=hgci, in1=ebuf, op=ALU.add)
                    pw_t = pwx_t if img == 0 else pwy_t
                    pso = small_ps()
                    for ci in range(2):
                        nc.tensor.matmul(pso[0:2, 0:64], pw_t[:, ci, :], hgci[:, ci, :],
                                         start=(ci == 0), stop=(ci == 1))
                    nc.vector.tensor_tensor(out=pos_sb[:, img, :], in0=pso[0:2, 0:64], in1=ref_t, op=ALU.add)
                    nc.vector.tensor_scalar(out=pos_sb[:, img, :], in0=pos_sb[:, img, :],
                                            scalar1=-1.0, scalar2=1.0, op0=ALU.max, op1=ALU.min)

                # transpose [2, 128] -> [128, 2] to get per-loc (y,x)
                psT = small_ps()
                nc.tensor.transpose(psT[:, 0:2], pos_sb.rearrange("p a b -> p (a b)"), identf)
                pos_pt = offp.tile([128, 2], f32)
                nc.vector.tensor_copy(pos_pt, psT[:, 0:2])

                # ----- pixel coords, floor, weights, indices -----
                pix = offp.tile([128, 2], f32)
                nc.vector.tensor_scalar(out=pix, in0=pos_pt, scalar1=1.0, scalar2=31.5,
                                        op0=ALU.add, op1=ALU.mult)
                ri = offp.tile([128, 2], i32)
                nc.vector.tensor_copy(ri, pix)
                rf = offp.tile([128, 2], f32)
                nc.vector.tensor_copy(rf, ri)
                gt = offp.tile([128, 2], f32)
                nc.vector.tensor_tensor(out=gt, in0=rf, in1=pix, op=ALU.is_gt)
                base = offp.tile([128, 2], f32)
                nc.vector.tensor_tensor(out=base, in0=rf, in1=gt, op=ALU.subtract)
                wf = offp.tile([128, 2], f32)
                nc.vector.tensor_tensor(out=wf, in0=pix, in1=base, op=ALU.subtract)
                y1x1 = offp.tile([128, 2], f32)
                nc.vector.tensor_scalar(out=y1x1, in0=base, scalar1=1.0, scalar2=63.0,
                                        op0=ALU.add, op1=ALU.min)
                omw = offp.tile([128, 2], f32)
                nc.vector.tensor_scalar(out=omw, in0=wf, scalar1=-1.0, scalar2=1.0,
                                        op0=ALU.mult, op1=ALU.add)
                wq = offp.tile([128, 4], f32)    # w00, w01, w10, w11
                nc.vector.tensor_tensor(out=wq[:, 0:1], in0=omw[:, 1:2], in1=omw[:, 0:1], op=ALU.mult)
                nc.vector.tensor_tensor(out=wq[:, 1:2], in0=wf[:, 1:2], in1=omw[:, 0:1], op=ALU.mult)
                nc.vector.tensor_tensor(out=wq[:, 2:3], in0=omw[:, 1:2], in1=wf[:, 0:1], op=ALU.mult)
                nc.vector.tensor_tensor(out=wq[:, 3:4], in0=wf[:, 1:2], in1=wf[:, 0:1], op=ALU.mult)
                idxf = offp.tile([128, 4], f32)
                nc.vector.tensor_scalar(out=idxf[:, 0:1], in0=base[:, 0:1], scalar1=72.0,
                                        scalar2=292.0, op0=ALU.mult, op1=ALU.add)
                nc.vector.tensor_tensor(out=idxf[:, 0:1], in0=idxf[:, 0:1], in1=base[:, 1:2], op=ALU.add)
                nc.vector.tensor_scalar(out=idxf[:, 1:2], in0=y1x1[:, 0:1], scalar1=72.0,
                                        scalar2=292.0, op0=ALU.mult, op1=ALU.add)
                nc.vector.tensor_tensor(out=idxf[:, 1:2], in0=idxf[:, 1:2], in1=base[:, 1:2], op=ALU.add)
                nc.vector.tensor_scalar(out=idxf[:, 2:3], in0=base[:, 0:1], scalar1=64.0,
                                        scalar2=None, op0=ALU.mult)
                nc.vector.tensor_tensor(out=idxf[:, 2:3], in0=idxf[:, 2:3], in1=base[:, 1:2], op=ALU.add)
                nc.vector.tensor_scalar(out=idxf[:, 3:4], in0=y1x1[:, 0:1], scalar1=64.0,
                                        scalar2=None, op0=ALU.mult)
                nc.vector.tensor_tensor(out=idxf[:, 3:4], in0=idxf[:, 3:4], in1=base[:, 1:2], op=ALU.add)
                idxi = offp.tile([128, 4], i32)
                nc.vector.tensor_copy(idxi, idxf)

                # ----- strip gathers + bilinear (bf16) -----
                def strip_gather(table, col):
                    g = gthp.tile([128, 512], bf16, tag="strip")
                    nc.gpsimd.indirect_dma_start(
                        out=g[:], out_offset=None, in_=table,
                        in_offset=bass.IndirectOffsetOnAxis(ap=idxi[:, col:col + 1], axis=0))
                    return g

                def bilin(g0, g1, name):
                    o = offp.tile([128, 256], bf16, name=name)
                    tmp = offp.tile([128, 256], bf16, tag="btmp")
                    nc.vector.tensor_scalar(out=o, in0=g0[:, 0:256], scalar1=wq[:, 0:1], scalar2=None, op0=ALU.mult)
                    nc.vector.tensor_scalar(out=tmp, in0=g0[:, 256:512], scalar1=wq[:, 1:2], scalar2=None, op0=ALU.mult)
                    nc.vector.tensor_tensor(out=o, in0=o, in1=tmp, op=ALU.add)
                    nc.vector.tensor_scalar(out=tmp, in0=g1[:, 0:256], scalar1=wq[:, 2:3], scalar2=None, op0=ALU.mult)
                    nc.vector.tensor_tensor(out=o, in0=o, in1=tmp, op=ALU.add)
                    nc.vector.tensor_scalar(out=tmp, in0=g1[:, 256:512], scalar1=wq[:, 3:4], scalar2=None, op0=ALU.mult)
                    nc.vector.tensor_tensor(out=o, in0=o, in1=tmp, op=ALU.add)
                    return o

                xsT = bilin(strip_gather(xT16[:], 0), strip_gather(xT16[:], 1), "xsT")
                ysT = bilin(strip_gather(yT16[:], 0), strip_gather(yT16[:], 1), "ysT")
                hsT = bilin(strip_gather(hT_d[:], 2), strip_gather(hT_d[:], 3), "hsT")

                # ----- sw branch -----
                hs = offp.tile([128, 2, 128], bf16)
                tph = tp_ps.tile([128, 512], bf16, tag="tp4")
                for ci in range(2):
                    nc.tensor.transpose(tph[:, ts(ci, 128)], hsT[:, ts(ci, 128)], ident16)
                nc.vector.tensor_copy(hs.rearrange("p a b -> p (a b)"), tph[:, 0:256])
                zr = offp.tile([128, 2, 128], bf16)
                for mo in range(2):
                    ps = small_ps()
                    for ci in range(2):
                        nc.tensor.matmul(ps, m1w_t[:, ci * 2 + mo, :], hs[:, ci, :],
                                         start=(ci == 0), stop=(ci == 1))
                    nc.scalar.activation(out=zr[:, mo, :], in_=ps, func=AF.Relu,
                                         bias=c1b_t[:, mo:mo + 1], scale=1.0)
                psS = small_ps()
                for ci in range(2):
                    nc.tensor.matmul(psS[0:2, :], sw2w_t[:, ci, :], zr[:, ci, :],
                                     start=(ci == 0), stop=(ci == 1))
                S_sb = offp.tile([2, 128], f32)
                nc.vector.tensor_copy(S_sb, psS[0:2, :])
                psT2 = small_ps()
                nc.tensor.transpose(psT2[:, 0:2], S_sb, identf)
                ST = offp.tile([128, 2], f32)
                nc.vector.tensor_copy(ST, psT2[:, 0:2])
                dS = offp.tile([128, 1], f32)
                nc.vector.tensor_tensor(out=dS, in0=ST[:, 0:1], in1=ST[:, 1:2], op=ALU.subtract)
                sw0 = offp.tile([128, 1], f32)
                nc.scalar.activation(out=sw0, in_=dS, func=AF.Sigmoid, bias=sigb_t[:, 0:1], scale=1.0)
                sw1 = offp.tile([128, 1], f32)
                nc.scalar.activation(out=sw1, in_=dS, func=AF.Sigmoid, bias=sigb_t[:, 1:2], scale=-1.0)

                # ----- sampled mix + k/v proj + vT8 -----
                smT = offp.tile([128, 256], bf16)
                tmp2 = offp.tile([128, 256], bf16)
                nc.vector.tensor_scalar(out=smT, in0=xsT, scalar1=sw0, scalar2=None, op0=ALU.mult)
                nc.vector.tensor_scalar(out=tmp2, in0=ysT, scalar1=sw1, scalar2=None, op0=ALU.mult)
                nc.vector.tensor_tensor(out=smT, in0=smT, in1=tmp2, op=ALU.add)
                smpl = offp.tile([128, 2, 128], bf16)
                tps = tp_ps.tile([128, 512], bf16, tag="tp4")
                for ci in range(2):
                    nc.tensor.transpose(tps[:, ts(ci, 128)], smT[:, ts(ci, 128)], ident16)
                nc.vector.tensor_copy(smpl.rearrange("p a b -> p (a b)"), tps[:, 0:256])
                v_t = offp.tile([128, 2, 128], bf16)
                for dst, wt, bt in ((k_t, pkw_t, pkb_t), (v_t, pvw_t, pvb_t)):
                    for mo in range(2):
                        ps = small_ps()
                        for ci in range(2):
                            nc.tensor.matmul(ps, wt[:, ci * 2 + mo, :], smpl[:, ci, :],
                                             start=(ci == 0), stop=(ci == 1))
                        nc.vector.tensor_scalar(out=dst[:, mo, :], in0=ps,
                                                scalar1=bt[:, mo:mo + 1], scalar2=None, op0=ALU.add)
                tpv = tp_ps.tile([128, 512], bf16, tag="tp4")
                for ci in range(2):
                    nc.tensor.transpose(tpv[:, ts(ci, 128)], v_t[:, ci, :], ident16)
                nc.vector.tensor_copy(vT8.rearrange("p a b -> p (a b)"), tpv[:, 0:256])

            # =======================================================
            # Phase B: attention + output projection
            # =======================================================
            with tc.tile_pool(name="apool", bufs=1) as apool, \
                 tc.tile_pool(name="epool", bufs=3) as epool, \
                 tc.tile_pool(name="npool", bufs=2) as npool, \
                 tc.tile_pool(name="qk_ps", bufs=3, space="PSUM") as qk_ps, \
                 tc.tile_pool(name="avs_ps", bufs=3, space="PSUM") as avs_ps, \
                 tc.tile_pool(name="po_ps", bufs=2, space="PSUM") as po_ps:

                att_t = apool.tile([128, 2, HW], bf16)
                E_tiles = {}

                def stage_qk(nb):
                    E = epool.tile([128, 8, 512], bf16, tag="E")
                    for hg4 in range(2):
                        for j in range(4):
                            qk = qk_ps.tile([128, 512], f32, tag="qk")
                            nc.tensor.matmul(qk, k_t[ts(j, 32), hg4, :],
                                             q_t[ts(j, 32), hg4, ts(nb, 512)],
                                             start=True, stop=True,
                                             tile_position=(32 * j, 0))
                            nc.scalar.activation(out=E[:, hg4 * 4 + j, :], in_=qk,
                                                 func=AF.Exp, scale=SCALE)
                    E_tiles[nb] = E

                def stage_av(nb):
                    E = E_tiles.pop(nb)
                    for g in range(2):
                        avg = avs_ps.tile([128, 512], f32, tag="avs")
                        for j in range(4):
                            nc.tensor.matmul(avg[ts(j, 32), :], vT8[:, g * 4 + j, :],
                                             E[:, g * 4 + j, :], start=True, stop=True,
                                             tile_position=(0, 32 * j))
                        sums = avs_ps.tile([128, 512], f32, tag="avs")
                        for j in range(4):
                            nc.tensor.matmul(sums[ts(j, 32), :], ones_m,
                                             E[:, g * 4 + j, :], start=True, stop=True,
                                             tile_position=(0, 32 * j))
                        rec = npool.tile([128, 512], f32, tag="rec")
                        nc.vector.reciprocal(out=rec, in_=sums)
                        nc.vector.tensor_tensor(out=att_t[:, g, ts(nb, 512)],
                                                in0=avg, in1=rec, op=ALU.mult)

                def stage_po(nb):
                    for mo in range(2):
                        ps = po_ps.tile([128, 512], f32, tag="po")
                        for ci in range(2):
                            nc.tensor.matmul(ps, pow_t[:, ci * 2 + mo, :],
                                             att_t[:, ci, ts(nb, 512)],
                                             start=(ci == 0), stop=False,
                                             skip_group_check=True)
                        nc.tensor.matmul(ps, pobr_t[0:1, mo, :], ones_bf,
                                         start=False, stop=True, skip_group_check=True)
                        ot = npool.tile([128, 512], f32, tag="ot")
                        nc.vector.tensor_copy(ot, ps)
                        nc.sync.dma_start(out=out_d[ts(mo, 128), ts(nb, 512)], in_=ot)

                for step in range(10):
                    if step < 8:
                        stage_qk(step)
                    if 1 <= step <= 8:
                        stage_av(step - 1)
                    if step >= 2:
                        stage_po(step - 2)

    nc.finalize()
    return nc


def _host_prep(inp):
    import ml_dtypes
    e4 = ml_dtypes.float8_e4m3
    bf = ml_dtypes.bfloat16
    g = {k: np.ascontiguousarray(np.asarray(v, dtype=np.float32)) for k, v in inp.items()}
    s = g['bn_g'] / np.sqrt(g['bn_v'] + EPS)
    fwf = g['fuse_w'] * s[:, None, None, None]          # [256, 512, 3, 3]
    fbf = (g['fuse_b'] - g['bn_m']) * s + g['bn_b']
    M1 = g['sw1_w'] @ g['projq_w']
    c1 = g['sw1_w'] @ g['projq_b'] + g['sw1_b']

    def lhsT4(wmat, dt=bf):  # [out, in] -> [128, 4(ci*2+mo), 128]
        a = np.zeros((128, 4, 128), np.float32)
        for ci in range(2):
            for mo in range(2):
                a[:, ci * 2 + mo, :] = wmat[mo * 128:(mo + 1) * 128, ci * 128:(ci + 1) * 128].T
        return a.astype(dt)

    def b2(vec):  # [256] -> [128, 2]
        return np.stack([vec[0:128], vec[128:256]], 1).astype(np.float32)

    d = {}
    if CONV_FP8:
        fw8 = np.zeros((128, 9, 2, 2, 2, 128), np.float32)
        for ky in range(3):
            for kx in range(3):
                tap = ky * 3 + kx
                for img in range(2):
                    for mo in range(2):
                        for i in range(2):
                            ci = img * 2 + i
                            fw8[:, tap, img, mo, i, :] = \
                                fwf[mo * 128:(mo + 1) * 128, ci * 128:(ci + 1) * 128, ky, kx].T * WS
        d['fw8'] = fw8.astype(e4).view(np.uint8)
    else:
        fw16 = np.zeros((128, 9, 4, 2, 128), np.float32)
        for ky in range(3):
            for kx in range(3):
                tap = ky * 3 + kx
                for ci in range(4):
                    for mo in range(2):
                        fw16[:, tap, ci, mo, :] = \
                            fwf[mo * 128:(mo + 1) * 128, ci * 128:(ci + 1) * 128, ky, kx].T
        d['fw16'] = fw16.astype(bf)
        d['fw8'] = np.zeros((128, 9, 2, 2, 2, 128), np.uint8)
    if not CONV_FP8:
        pass
    d['fb'] = b2(fbf)
    d['pqw'] = lhsT4(g['projq_w']); d['pqb'] = b2(g['projq_b'])
    d['m1w'] = lhsT4(M1);           d['c1b'] = b2(c1)
    sw2a = np.zeros((128, 2, 2), np.float32)
    for ci in range(2):
        sw2a[:, ci, :] = g['sw2_w'][:, ci * 128:(ci + 1) * 128].T
    d['sw2w'] = sw2a.astype(bf)
    db = float(g['sw2_b'][0] - g['sw2_b'][1])
    d['sigb'] = np.tile(np.array([[db, -db]], np.float32), (128, 1))
    for nm, key in (('pwx', 'offx_pw_w'), ('pwy', 'offy_pw_w')):
        a = np.zeros((128, 2, 2), np.float32)
        for ci in range(2):
            a[:, ci, :] = 0.5 * g[key][:, ci * 128:(ci + 1) * 128].T
        d[nm] = a
    dwsc = np.zeros((128, 2, 2, 81), np.float32)
    dwbc = np.zeros((128, 2, 2), np.float32)
    lnGc = np.zeros((128, 2, 2), np.float32)
    lnBc = np.zeros((128, 2, 2), np.float32)
    for img, pre in ((0, 'offx'), (1, 'offy')):
        w = g[pre + '_dw_w'][:, 0].reshape(256, 81)
        for ci in range(2):
            dwsc[:, ci, img, :] = w[ci * 128:(ci + 1) * 128]
            dwbc[:, img, ci] = g[pre + '_dw_b'][ci * 128:(ci + 1) * 128]
            lnGc[:, img, ci] = g[pre + '_ln_g'][ci * 128:(ci + 1) * 128]
            lnBc[:, img, ci] = g[pre + '_ln_b'][ci * 128:(ci + 1) * 128]
    d['dwsc'] = dwsc.astype(np.float16)
    d['dwbc'] = dwbc; d['lnGc'] = lnGc; d['lnBc'] = lnBc
    ry = (np.linspace(0.5, Hk - 0.5, Hk, dtype=np.float32) / np.float32(Hk - 1.0)) * 2.0 - 1.0
    gy, gx = np.meshgrid(ry, ry, indexing='ij')
    d['ref2'] = np.stack([gy, gx], 0).reshape(2, 64).astype(np.float32)
    d['pkw'] = lhsT4(g['projk_w']); d['pkb'] = b2(g['projk_b'])
    d['pvw'] = lhsT4(g['projv_w']); d['pvb'] = b2(g['projv_b'])
    d['pow'] = lhsT4(g['projo_w'])
    pobr = np.zeros((1, 2, 128), np.float32)
    pobr[0, 0, :] = g['projo_b'][0:128]
    pobr[0, 1, :] = g['projo_b'][128:256]
    d['pobr'] = pobr.astype(bf)
    return g, d


def kernel(**inputs):
    import ml_dtypes
    from concourse.bass_utils import run_bass_kernel_spmd
    e4 = ml_dtypes.float8_e4m3
    bf = ml_dtypes.bfloat16

    if 'nc' not in _CACHE:
        _CACHE['nc'] = _build_program()
    nc = _CACHE['nc']

    g, wd = _host_prep(inputs)
    in_maps = []
    for b in range(B):
        m = dict(wd)
        for img, key in ((0, 'x'), (1, 'y')):
            im = g[key][b]                                    # [256, 64, 64]
            # shifted fp8 copies for the 3 horizontal taps
            xs = np.zeros((128, 3, 2, 4096), np.float32) if img == 0 else m['_xs_f']
            for ci in range(2):
                ch = im[ci * 128:(ci + 1) * 128]              # [128, 64, 64]
                sh0 = np.zeros_like(ch); sh0[:, :, 1:] = ch[:, :, :-1]
                sh2 = np.zeros_like(ch); sh2[:, :, :-1] = ch[:, :, 1:]
                xs[:, 0, ci, :] = sh0.reshape(128, 4096)
                xs[:, 1, ci, :] = ch.reshape(128, 4096)
                xs[:, 2, ci, :] = sh2.reshape(128, 4096)
            if img == 0:
                m['_xs_f'] = xs
                m['_xs_all'] = np.zeros((128, 3, 2, 2, 4096), np.float32)
            m['_xs_all'][:, :, img, :, :] = xs
            # fp16 padded dwconv input [128, img, ci, 5184]
            if img == 0:
                m['_dwp'] = np.zeros((128, 2, 2, PADR, PADR), np.float16)
            for ci in range(2):
                m['_dwp'][:, img, ci, 4:68, 4:68] = im[ci * 128:(ci + 1) * 128]
            # bf16 gather table [5184, 256]
            t = np.zeros((PADR, PADR, C), np.float32)
            t[4:68, 4:68] = im.transpose(1, 2, 0)
            m['xT16' if img == 0 else 'yT16'] = t.reshape(NROW, C).astype(bf)
        if CONV_FP8:
            m['xs8'] = m['_xs_all'].astype(e4).view(np.uint8)
            m['xs16'] = None
        else:
            m['xs16'] = m['_xs_all'].astype(bf)
            m['xs8'] = np.zeros((128, 3, 2, 2, 4096), np.uint8)
        m['dwp16'] = m.pop('_dwp').reshape(128, 2, 2, NROW)
        m.pop('_xs_f'); m.pop('_xs_all')
        if m.get('xs16') is None:
            m.pop('xs16', None)
        in_maps.append(m)

    res = run_bass_kernel_spmd(nc, in_maps, list(range(B)))
    out = np.stack([res.results[i]['out'].reshape(C, H, W) for i in range(B)])
    return out.astype(np.float32)


# revision 13
# speedup vs baseline: 2.1108x; 1.1422x over previous
"""Trainium2 Bass kernel for nn_DAttentionMM (deformable attention, multi-modal).

Strategy: data-parallel over batch B=8 across 8 NeuronCores. Each core runs the
full per-batch pipeline:
  conv3x3 (fp8e4m3 DoubleRow matmuls, 2x128-channel contraction per instr, on a
  72-padded input) + GELU -> q proj (bf16) -> offset branch (fp16 dwconv on DVE,
  LN stats via ones-matmuls, quake-rsqrt on DVE, native GELU) -> bilinear
  sampling of x, y (bf16 DRAM tables) and h (bf16 hT scratch) -> tanh-based sw
  mixing -> k/v proj -> 8-head attention in bf16 (packed 4-head AV psum
  quadrants, ones-matmul softmax sums) -> output proj, bias added in the psum
  drain copy.

The offset-branch mid-section is issued between conv blocks 6 and 7 so the
in-order ACT/DVE/Pool queues process it as soon as the dwconv data is ready.
Activation-table switches: gelu (initial) and exp (phase B) only.

Precision (validated host-side vs the jax reference, ~7e-3; measured 1.2e-2 on
HW): conv in/weights fp8e4m3 (weights pre-scaled x64, undone in the GELU),
offset branch fp16, sampling tables / q / k / v / E bf16, everything else f32.
"""
import sys

sys.path.insert(0, '/opt/trn_rl_repo')

import numpy as np

B, C, H, W = 8, 256, 64, 64
NH, HC = 8, 32
Hk = Wk = 8
NS = 64
SCALE = float(HC) ** -0.5
EPS = 1e-5
HW = H * W
PADR = 72          # padded rows/cols (+4 each side)
NROW = PADR * PADR  # 5184
WS = 64.0          # fp8 conv weight pre-scale (undone in the gelu activation)

CONV_FP8 = True    # fp8 DoubleRow conv; False = bf16 fallback (36 matmuls/psum)

_CACHE = {}


def _build_program():
    import concourse.bass as bass
    import concourse.tile as tile
    from concourse import bacc, mybir
    from concourse.masks import make_identity

    f32 = mybir.dt.float32
    i32 = mybir.dt.int32
    bf16 = mybir.dt.bfloat16
    f16 = mybir.dt.float16
    f8 = mybir.dt.float8e4
    u8 = mybir.dt.uint8
    AF = mybir.ActivationFunctionType
    ALU = mybir.AluOpType
    PM = mybir.MatmulPerfMode
    ts = bass.ts

    nc = bacc.Bacc("TRN2", target_bir_lowering=False, debug=False)

    dp = lambda name, shape, dt=f32: nc.declare_dram_parameter(name, list(shape), dt, isOutput=False)
    if CONV_FP8:
        xp8 = dp("xp8", (128, 2, 2, NROW), u8)        # [p, img, ci, 72*72] fp8
        fw8 = dp("fw8", (128, 9, 2, 2, 2, 128), u8)   # [p, tap, img, mo, i, m]
    else:
        xp16 = dp("xp16", (128, 2, 2, NROW), bf16)
        fw16 = dp("fw16", (128, 9, 4, 2, 128), bf16)  # [p, tap, ci, mo, m]
    dwp16 = dp("dwp16", (128, 2, 2, NROW), f16)       # [p, img, ci, 72*72]
    fb = dp("fb", (128, 2))
    pqw = dp("pqw", (128, 4, 128), bf16)              # [p, ci*2+mo, m]
    pqb = dp("pqb", (128, 2))
    m1w = dp("m1w", (128, 4, 128), bf16)
    c1b = dp("c1b", (128, 2))
    sw2w = dp("sw2w", (128, 2, 2), bf16)              # [p, ci, t]
    sigb = dp("sigb", (128, 1))                       # db/2 for the tanh trick
    pwx = dp("pwx", (128, 2, 2))                      # [p, ci, t]
    pwy = dp("pwy", (128, 2, 2))
    dwsc = dp("dwsc", (128, 2, 2, 81), f16)           # [p, ci, img, tap]
    dwbc = dp("dwbc", (128, 2, 2))                    # [p, img, ci]
    lnGc = dp("lnGc", (128, 2, 2))
    lnBc = dp("lnBc", (128, 2, 2))
    ref2 = dp("ref2", (2, 64))
    pkw = dp("pkw", (128, 4, 128), bf16)
    pkb = dp("pkb", (128, 2))
    pvw = dp("pvw", (128, 4, 128), bf16)
    pvb = dp("pvb", (128, 2))
    pow_ = dp("pow", (128, 4, 128), bf16)
    pob = dp("pob", (128, 2))
    xT16 = dp("xT16", (NROW, 256), bf16)
    yT16 = dp("yT16", (NROW, 256), bf16)

    out_d = nc.declare_dram_parameter("out", [C, HW], f32, isOutput=True)
    hT_d = nc.dram_tensor("hT_scratch", [HW + 1, 256], bf16)

    with tile.TileContext(nc) as tc:
        import contextlib
        with contextlib.ExitStack() as ctx:
            const = ctx.enter_context(tc.tile_pool(name="const", bufs=1))
            work = ctx.enter_context(tc.tile_pool(name="work", bufs=1))

            # ---------------- constants ----------------
            ident16 = const.tile([128, 128], bf16)
            make_identity(nc, ident16)
            identf = const.tile([2, 2], f32)
            make_identity(nc, identf)
            ones_m = const.tile([128, 32], bf16)
            nc.vector.memset(ones_m, 1.0)
            onesr = const.tile([128, 1], f32)
            nc.vector.memset(onesr, 1.0)
            zrow = const.tile([1, 256], bf16)
            nc.vector.memset(zrow, 0.0)

            # ---------------- DMA loads, in need-order ----------------
            # wave 1: conv inputs (conv starts ~11us in)
            if CONV_FP8:
                xp_t = const.tile([128, 2, 2, NROW], f8)
                fw_t = const.tile([128, 9, 2, 2, 2, 128], f8)
                nc.scalar.dma_start(out=fw_t, in_=fw8[:].bitcast(f8))
                nc.sync.dma_start(out=xp_t[:, 0], in_=xp8[:, 0].bitcast(f8))
                nc.sync.dma_start(out=xp_t[:, 1], in_=xp8[:, 1].bitcast(f8))
            else:
                xp_t = const.tile([128, 2, 2, NROW], bf16)
                fw_t = const.tile([128, 9, 4, 2, 128], bf16)
                nc.scalar.dma_start(out=fw_t, in_=fw16[:])
                nc.sync.dma_start(out=xp_t[:, 0], in_=xp16[:, 0])
                nc.sync.dma_start(out=xp_t[:, 1], in_=xp16[:, 1])
            # wave 2: small weights needed by the conv loop + offset branch
            _dn = [0]
            def ctile(shape, dt, srcap, eng):
                _dn[0] += 1
                t = const.tile(shape, dt, name=f"cw{_dn[0]}")
                eng.dma_start(out=t, in_=srcap)
                return t
            pqw_t = ctile([128, 4, 128], bf16, pqw[:], nc.scalar)
            pqb_t = ctile([128, 2], f32, pqb[:], nc.scalar)
            fb_t = ctile([128, 2], f32, fb[:], nc.scalar)
            dwsc_t = ctile([128, 2, 2, 81], f16, dwsc[:], nc.scalar)
            dwbc_t = ctile([128, 2, 2], f32, dwbc[:], nc.scalar)
            lnGc_t = ctile([128, 2, 2], f32, lnGc[:], nc.scalar)
            lnBc_t = ctile([128, 2, 2], f32, lnBc[:], nc.scalar)
            ref_t = ctile([2, 64], f32, ref2[:], nc.scalar)
            pwx_t = ctile([128, 2, 2], f32, pwx[:], nc.scalar)
            pwy_t = ctile([128, 2, 2], f32, pwy[:], nc.scalar)
            nc.scalar.dma_start(out=hT_d[HW:HW + 1, :], in_=zrow)
            # wave 3: dwconv input, 4 chunks in consumption order
            dwp_t = const.tile([128, 2, 2, NROW], f16)
            nc.sync.dma_start(out=dwp_t[:, 0, 0], in_=dwp16[:, 0, 0])
            nc.scalar.dma_start(out=dwp_t[:, 0, 1], in_=dwp16[:, 0, 1])
            nc.sync.dma_start(out=dwp_t[:, 1, 0], in_=dwp16[:, 1, 0])
            nc.scalar.dma_start(out=dwp_t[:, 1, 1], in_=dwp16[:, 1, 1])
            # wave 4: tail/phase-B weights
            m1w_t = ctile([128, 4, 128], bf16, m1w[:], nc.sync)
            c1b_t = ctile([128, 2], f32, c1b[:], nc.sync)
            sw2w_t = ctile([128, 2, 2], bf16, sw2w[:], nc.sync)
            sigb_t = ctile([128, 1], f32, sigb[:], nc.sync)
            pkw_t = ctile([128, 4, 128], bf16, pkw[:], nc.sync)
            pkb_t = ctile([128, 2], f32, pkb[:], nc.sync)
            pvw_t = ctile([128, 4, 128], bf16, pvw[:], nc.sync)
            pvb_t = ctile([128, 2], f32, pvb[:], nc.sync)
            pow_t = ctile([128, 4, 128], bf16, pow_[:], nc.sync)
            pob_t = ctile([128, 2], f32, pob[:], nc.sync)

            # persistent activations
            q_t = work.tile([128, 2, HW], bf16)
            k_t = work.tile([128, 2, 128], bf16)
            vT8 = work.tile([128, 8, 32], bf16)

            # =======================================================
            # Phase A
            # =======================================================
            with tc.tile_pool(name="dwp", bufs=2) as dwp, \
                 tc.tile_pool(name="offp", bufs=1) as offp, \
                 tc.tile_pool(name="hbp", bufs=2) as hbp, \
                 tc.tile_pool(name="stp", bufs=2) as stp, \
                 tc.tile_pool(name="gthp", bufs=2) as gthp, \
                 tc.tile_pool(name="conv_ps", bufs=2, space="PSUM") as conv_ps, \
                 tc.tile_pool(name="pq_ps", bufs=2, space="PSUM") as pq_ps, \
                 tc.tile_pool(name="tp_ps", bufs=2, space="PSUM") as tp_ps, \
                 tc.tile_pool(name="sm_ps", bufs=2, space="PSUM") as sm_ps:

                _spn = [0]
                def small_ps():
                    _spn[0] += 1
                    return sm_ps.tile([128, 128], f32, tag="smallf", name=f"sp{_spn[0]}")

                # ----- dwconv on DVE (fp16), issued first -----
                accs = {}
                for img in range(2):
                    for ci in range(2):
                        acc576 = dwp.tile([128, 576], f16, tag="a576")
                        tmp576 = dwp.tile([128, 576], f16, tag="t576")
                        dbase = dwp_t.offset + (img * 2 + ci) * NROW
                        for ky in range(9):
                            sl = bass.AP(tensor=dwp_t.tensor, offset=dbase + ky * 72,
                                         ap=[dwp_t.ap[0], [576, 8], [8, 8], [1, 9]])
                            wsl = dwsc_t[:, ci, img, ky * 9:(ky + 1) * 9]
                            wbc = bass.AP(tensor=wsl.tensor, offset=wsl.offset,
                                          ap=[wsl.ap[0], [0, 8], [0, 8], [1, 9]])
                            dst = acc576 if ky == 0 else tmp576
                            nc.vector.tensor_tensor(
                                out=dst[:, :].rearrange("p (a b c) -> p a b c", a=8, b=8),
                                in0=sl, in1=wbc, op=ALU.mult)
                            if ky > 0:
                                nc.vector.tensor_tensor(out=acc576, in0=acc576, in1=tmp576, op=ALU.add)
                        acc = offp.tile([128, 64], f32, name=f"dwacc{img}{ci}")
                        rview = bass.AP(tensor=acc576.tensor, offset=acc576.offset,
                                        ap=[acc576.ap[0], [9, 64], [1, 9]])
                        nc.vector.reduce_sum(out=acc, in_=rview, axis=mybir.AxisListType.X)
                        nc.vector.tensor_scalar(out=acc, in0=acc, scalar1=dwbc_t[:, img, ci:ci + 1],
                                                scalar2=None, op0=ALU.add)
                        accs[(img, ci)] = acc

                # ----- conv3x3 blocks -----
                def conv_block(rb):
                    hb = hbp.tile([128, 2, 512], bf16, tag="hblk")
                    for mo in range(2):
                        ps = conv_ps.tile([128, 512], f32, tag="cps")
                        first = True
                        nmm = 18 if CONV_FP8 else 36
                        done = 0
                        for ky in range(3):
                            for kx in range(3):
                                tap = ky * 3 + kx
                                off = (rb * 8 + ky + 3) * 72 + kx + 3
                                if CONV_FP8:
                                    for img in range(2):
                                        rhs = bass.AP(tensor=xp_t.tensor,
                                                      offset=xp_t.offset + img * 2 * NROW + off,
                                                      ap=[xp_t.ap[0], [NROW, 2], [72, 8], [1, 64]])
                                        done += 1
                                        nc.tensor.matmul(
                                            ps, fw_t[:, tap, img, mo], rhs,
                                            start=first, stop=(done == nmm),
                                            perf_mode=PM.DoubleRow,
                                            skip_group_check=True)
                                        first = False
                                else:
                                    for cidx in range(4):
                                        rhs = bass.AP(tensor=xp_t.tensor,
                                                      offset=xp_t.offset + cidx * NROW + off,
                                                      ap=[xp_t.ap[0], [72, 8], [1, 64]])
                                        done += 1
                                        nc.tensor.matmul(
                                            ps, fw_t[:, tap, cidx, mo], rhs,
                                            start=first, stop=(done == nmm),
                                            skip_group_check=True)
                                        first = False
                        nc.scalar.activation(out=hb[:, mo, :], in_=ps, func=AF.Gelu,
                                             bias=fb_t[:, mo:mo + 1],
                                             scale=(1.0 / WS) if CONV_FP8 else 1.0)
                    return hb

                def q_and_transpose(rb, hb):
                    for mo in range(2):
                        ps = pq_ps.tile([128, 512], f32, tag="pqs")
                        for ci in range(2):
                            nc.tensor.matmul(ps, pqw_t[:, ci * 2 + mo, :], hb[:, ci, :],
                                             start=(ci == 0), stop=(ci == 1))
                        nc.scalar.activation(out=q_t[:, mo, ts(rb, 512)], in_=ps,
                                             func=AF.Identity, bias=pqb_t[:, mo:mo + 1], scale=1.0)
                    stg = stp.tile([128, 4, 2, 128], bf16, tag="hstage")
                    for ci in range(2):
                        tp = tp_ps.tile([128, 512], bf16, tag="tp4")
                        for s4 in range(4):
                            nc.tensor.transpose(tp[:, ts(s4, 128)], hb[:, ci, ts(s4, 128)],
                                                ident16)
                        nc.scalar.activation(
                            out=bass.AP(tensor=stg.tensor, offset=stg.offset + ci * 128,
                                        ap=[stg.ap[0], [256, 4], [1, 128]]),
                            in_=tp, func=AF.Copy, bias=0.0, scale=1.0)
                    nc.sync.dma_start(out=hT_d[rb * 512:(rb + 1) * 512, :]
                                      .rearrange("(s p) c -> p s c", p=128),
                                      in_=stg)

                # mid-section state shared with the tail
                mid = {}

                def offset_mid_section():
                    # LN stats via ones-matmuls
                    var2 = offp.tile([1, 2, 64], f32)
                    mean_ = offp.tile([1, 2, 64], f32)
                    for img in range(2):
                        sqs = []
                        for ci in range(2):
                            sq = dwp.tile([128, 64], f32, tag="dwsq")
                            nc.vector.tensor_tensor(out=sq, in0=accs[(img, ci)],
                                                    in1=accs[(img, ci)], op=ALU.mult)
                            sqs.append(sq)
                        ps_s1 = small_ps()
                        for ci in range(2):
                            nc.tensor.matmul(ps_s1[0:1, 0:64], onesr, accs[(img, ci)],
                                             start=(ci == 0), stop=(ci == 1))
                        ps_s2 = small_ps()
                        for ci in range(2):
                            nc.tensor.matmul(ps_s2[0:1, 0:64], onesr, sqs[ci],
                                             start=(ci == 0), stop=(ci == 1))
                        nc.vector.tensor_scalar(out=mean_[:, img, :], in0=ps_s1[0:1, 0:64],
                                                scalar1=1.0 / 256.0, scalar2=None, op0=ALU.mult)
                        ex2 = offp.tile([1, 64], f32, tag="ex2")
                        nc.vector.tensor_scalar(out=ex2, in0=ps_s2[0:1, 0:64],
                                                scalar1=1.0 / 256.0, scalar2=None, op0=ALU.mult)
                        msq = offp.tile([1, 64], f32, tag="msq")
                        nc.vector.tensor_tensor(out=msq, in0=mean_[:, img, :], in1=mean_[:, img, :], op=ALU.mult)
                        nc.vector.tensor_tensor(out=var2[:, img, :], in0=ex2, in1=msq, op=ALU.subtract)
                        nc.vector.tensor_scalar(out=var2[:, img, :], in0=var2[:, img, :],
                                                scalar1=EPS, scalar2=None, op0=ALU.add)
                    # rstd = rsqrt(var) via quake + 3 Newton iters, all on DVE
                    vflat = var2.rearrange("a b c -> a (b c)")
                    rstd2 = offp.tile([1, 2, 64], f32)
                    rflat = rstd2.rearrange("a b c -> a (b c)")
                    qi = offp.tile([1, 128], i32)
                    nc.vector.tensor_scalar(out=qi, in0=vflat.bitcast(i32), scalar1=1,
                                            scalar2=None, op0=ALU.logical_shift_right)
                    nc.vector.tensor_scalar(out=qi, in0=qi, scalar1=-1, scalar2=0x5f3759df,
                                            op0=ALU.mult, op1=ALU.add)
                    qt_ = offp.tile([1, 128], f32)
                    y0 = qi.bitcast(f32)
                    nc.vector.tensor_tensor(out=qt_, in0=y0, in1=y0, op=ALU.mult)
                    nc.vector.tensor_tensor(out=qt_, in0=qt_, in1=vflat, op=ALU.mult)
                    nc.vector.tensor_scalar(out=qt_, in0=qt_, scalar1=-0.5, scalar2=1.5,
                                            op0=ALU.mult, op1=ALU.add)
                    nc.vector.tensor_tensor(out=rflat, in0=y0, in1=qt_, op=ALU.mult)
                    for _ in range(2):
                        nc.vector.tensor_tensor(out=qt_, in0=rflat, in1=rflat, op=ALU.mult)
                        nc.vector.tensor_tensor(out=qt_, in0=qt_, in1=vflat, op=ALU.mult)
                        nc.vector.tensor_scalar(out=qt_, in0=qt_, scalar1=-0.5, scalar2=1.5,
                                                op0=ALU.mult, op1=ALU.add)
                        nc.vector.tensor_tensor(out=rflat, in0=rflat, in1=qt_, op=ALU.mult)

                    pos_sb = offp.tile([2, 2, 64], f32)   # [yx, grid, s]
                    for img in range(2):
                        mbc = offp.tile([128, 64], f32, tag="mbc")
                        nc.gpsimd.partition_broadcast(mbc[:], mean_[0:1, img, :])
                        rbc = offp.tile([128, 64], f32, tag="rbc")
                        nc.gpsimd.partition_broadcast(rbc[:], rstd2[0:1, img, :])
                        hgci = offp.tile([128, 2, 64], f32, name=f"hgc_{img}")
                        for ci in range(2):
                            t2 = dwp.tile([128, 64], f32, tag="dwt2")
                            nc.vector.tensor_tensor(out=t2, in0=accs[(img, ci)], in1=mbc, op=ALU.subtract)
                            nc.vector.tensor_tensor(out=t2, in0=t2, in1=rbc, op=ALU.mult)
                            nc.vector.tensor_scalar(out=t2, in0=t2, scalar1=lnGc_t[:, img, ci:ci + 1],
                                                    scalar2=None, op0=ALU.mult)
                            nc.vector.tensor_scalar(out=t2, in0=t2,
                                                    scalar1=lnBc_t[:, img, ci:ci + 1],
                                                    scalar2=None, op0=ALU.add)
                            nc.scalar.activation(out=hgci[:, ci, :], in_=t2, func=AF.Gelu, scale=1.0)
                        pw_t = pwx_t if img == 0 else pwy_t
                        pso = small_ps()
                        for ci in range(2):
                            nc.tensor.matmul(pso[0:2, 0:64], pw_t[:, ci, :], hgci[:, ci, :],
                                             start=(ci == 0), stop=(ci == 1))
                        nc.vector.tensor_tensor(out=pos_sb[:, img, :], in0=pso[0:2, 0:64],
                                                in1=ref_t, op=ALU.add)
                        nc.vector.tensor_scalar(out=pos_sb[:, img, :], in0=pos_sb[:, img, :],
                                                scalar1=-1.0, scalar2=1.0, op0=ALU.max, op1=ALU.min)

                    psT = small_ps()
                    nc.tensor.transpose(psT[:, 0:2], pos_sb.rearrange("p a b -> p (a b)"), identf)
                    pos_pt = offp.tile([128, 2], f32)
                    nc.vector.tensor_copy(pos_pt, psT[:, 0:2])

                    pix = offp.tile([128, 2], f32)
                    nc.vector.tensor_scalar(out=pix, in0=pos_pt, scalar1=1.0, scalar2=31.5,
                                            op0=ALU.add, op1=ALU.mult)
                    ri = offp.tile([128, 2], i32)
                    nc.vector.tensor_copy(ri, pix)
                    rf = offp.tile([128, 2], f32)
                    nc.vector.tensor_copy(rf, ri)
                    gt = offp.tile([128, 2], f32)
                    nc.vector.tensor_tensor(out=gt, in0=rf, in1=pix, op=ALU.is_gt)
                    fbase = offp.tile([128, 2], f32)
                    nc.vector.tensor_tensor(out=fbase, in0=rf, in1=gt, op=ALU.subtract)
                    wf = offp.tile([128, 2], f32)
                    nc.vector.tensor_tensor(out=wf, in0=pix, in1=fbase, op=ALU.subtract)
                    y1x1 = offp.tile([128, 2], f32)
                    nc.vector.tensor_scalar(out=y1x1, in0=fbase, scalar1=1.0, scalar2=63.0,
                                            op0=ALU.add, op1=ALU.min)
                    omw = offp.tile([128, 2], f32)
                    nc.vector.tensor_scalar(out=omw, in0=wf, scalar1=-1.0, scalar2=1.0,
                                            op0=ALU.mult, op1=ALU.add)
                    wq = offp.tile([128, 4], f32)    # w00, w01, w10, w11
                    nc.vector.tensor_tensor(out=wq[:, 0:1], in0=omw[:, 1:2], in1=omw[:, 0:1], op=ALU.mult)
                    nc.vector.tensor_tensor(out=wq[:, 1:2], in0=wf[:, 1:2], in1=omw[:, 0:1], op=ALU.mult)
                    nc.vector.tensor_tensor(out=wq[:, 2:3], in0=omw[:, 1:2], in1=wf[:, 0:1], op=ALU.mult)
                    nc.vector.tensor_tensor(out=wq[:, 3:4], in0=wf[:, 1:2], in1=wf[:, 0:1], op=ALU.mult)
                    idxf = offp.tile([128, 4], f32)
                    nc.vector.tensor_scalar(out=idxf[:, 0:1], in0=fbase[:, 0:1], scalar1=72.0,
                                            scalar2=292.0, op0=ALU.mult, op1=ALU.add)
                    nc.vector.tensor_tensor(out=idxf[:, 0:1], in0=idxf[:, 0:1], in1=fbase[:, 1:2], op=ALU.add)
                    nc.vector.tensor_scalar(out=idxf[:, 1:2], in0=y1x1[:, 0:1], scalar1=72.0,
                                            scalar2=292.0, op0=ALU.mult, op1=ALU.add)
                    nc.vector.tensor_tensor(out=idxf[:, 1:2], in0=idxf[:, 1:2], in1=fbase[:, 1:2], op=ALU.add)
                    nc.vector.tensor_scalar(out=idxf[:, 2:3], in0=fbase[:, 0:1], scalar1=64.0,
                                            scalar2=None, op0=ALU.mult)
                    nc.vector.tensor_tensor(out=idxf[:, 2:3], in0=idxf[:, 2:3], in1=fbase[:, 1:2], op=ALU.add)
                    nc.vector.tensor_scalar(out=idxf[:, 3:4], in0=y1x1[:, 0:1], scalar1=64.0,
                                            scalar2=None, op0=ALU.mult)
                    nc.vector.tensor_tensor(out=idxf[:, 3:4], in0=idxf[:, 3:4], in1=fbase[:, 1:2], op=ALU.add)
                    idxi = offp.tile([128, 4], i32)
                    nc.vector.tensor_copy(idxi, idxf)
                    mid['idxi'] = idxi
                    mid['wq'] = wq

                    # x/y gathers + bilins run during the last conv block
                    mid['xsT'] = bilin(strip_gather(xT16[:], 0), strip_gather(xT16[:], 1), "xsT")
                    mid['ysT'] = bilin(strip_gather(yT16[:], 0), strip_gather(yT16[:], 1), "ysT")

                def strip_gather(table, col):
                    g = gthp.tile([128, 512], bf16, tag="strip")
                    nc.gpsimd.indirect_dma_start(
                        out=g[:], out_offset=None, in_=table,
                        in_offset=bass.IndirectOffsetOnAxis(ap=mid['idxi'][:, col:col + 1], axis=0))
                    return g

                def bilin(g0, g1, name):
                    wq = mid['wq']
                    o = offp.tile([128, 256], bf16, name=name)
                    tmp = offp.tile([128, 256], bf16, tag="btmp")
                    nc.vector.tensor_scalar(out=o, in0=g0[:, 0:256], scalar1=wq[:, 0:1], scalar2=None, op0=ALU.mult)
                    nc.vector.tensor_scalar(out=tmp, in0=g0[:, 256:512], scalar1=wq[:, 1:2], scalar2=None, op0=ALU.mult)
                    nc.vector.tensor_tensor(out=o, in0=o, in1=tmp, op=ALU.add)
                    nc.vector.tensor_scalar(out=tmp, in0=g1[:, 0:256], scalar1=wq[:, 2:3], scalar2=None, op0=ALU.mult)
                    nc.vector.tensor_tensor(out=o, in0=o, in1=tmp, op=ALU.add)
                    nc.vector.tensor_scalar(out=tmp, in0=g1[:, 256:512], scalar1=wq[:, 3:4], scalar2=None, op0=ALU.mult)
                    nc.vector.tensor_tensor(out=o, in0=o, in1=tmp, op=ALU.add)
                    return o

                # conv blocks 0..6, mid-section, block 7
                hb_prev = None
                for rb in range(7):
                    hb = conv_block(rb)
                    if hb_prev is not None:
                        q_and_transpose(rb - 1, hb_prev)
                    hb_prev = hb
                offset_mid_section()
                hb = conv_block(7)
                q_and_transpose(6, hb_prev)
                q_and_transpose(7, hb)

                # ----- h sampling (needs full hT) -----
                hsT = bilin(strip_gather(hT_d[:], 2), strip_gather(hT_d[:], 3), "hsT")

                # ----- sw branch (tanh-based sigmoid, no table switch) -----
                hs = offp.tile([128, 2, 128], bf16)
                tph = tp_ps.tile([128, 512], bf16, tag="tp4")
                for ci in range(2):
                    nc.tensor.transpose(tph[:, ts(ci, 128)], hsT[:, ts(ci, 128)], ident16)
                nc.vector.tensor_copy(hs.rearrange("p a b -> p (a b)"), tph[:, 0:256])
                zr = offp.tile([128, 2, 128], bf16)
                for mo in range(2):
                    ps = small_ps()
                    for ci in range(2):
                        nc.tensor.matmul(ps, m1w_t[:, ci * 2 + mo, :], hs[:, ci, :],
                                         start=(ci == 0), stop=(ci == 1))
                    nc.scalar.activation(out=zr[:, mo, :], in_=ps, func=AF.Relu,
                                         bias=c1b_t[:, mo:mo + 1], scale=1.0)
                psS = small_ps()
                for ci in range(2):
                    nc.tensor.matmul(psS[0:2, :], sw2w_t[:, ci, :], zr[:, ci, :],
                                     start=(ci == 0), stop=(ci == 1))
                S_sb = offp.tile([2, 128], f32)
                nc.vector.tensor_copy(S_sb, psS[0:2, :])
                psT2 = small_ps()
                nc.tensor.transpose(psT2[:, 0:2], S_sb, identf)
                ST = offp.tile([128, 2], f32)
                nc.vector.tensor_copy(ST, psT2[:, 0:2])
                dS = offp.tile([128, 1], f32)
                nc.vector.tensor_tensor(out=dS, in0=ST[:, 0:1], in1=ST[:, 1:2], op=ALU.subtract)
                # sigmoid(dS+db) = 0.5 + 0.5*tanh((dS+db)/2); tanh is in the gelu table
                th = offp.tile([128, 1], f32)
                nc.scalar.activation(out=th, in_=dS, func=AF.Tanh, bias=sigb_t[:, 0:1], scale=0.5)
                sw0 = offp.tile([128, 1], f32)
                nc.vector.tensor_scalar(out=sw0, in0=th, scalar1=0.5, scalar2=0.5,
                                        op0=ALU.mult, op1=ALU.add)
                sw1 = offp.tile([128, 1], f32)
                nc.vector.tensor_scalar(out=sw1, in0=th, scalar1=-0.5, scalar2=0.5,
                                        op0=ALU.mult, op1=ALU.add)

                # ----- sampled mix + k/v proj + vT8 -----
                smT = offp.tile([128, 256], bf16)
                tmp2 = offp.tile([128, 256], bf16)
                nc.vector.tensor_scalar(out=smT, in0=mid['xsT'], scalar1=sw0, scalar2=None, op0=ALU.mult)
                nc.vector.tensor_scalar(out=tmp2, in0=mid['ysT'], scalar1=sw1, scalar2=None, op0=ALU.mult)
                nc.vector.tensor_tensor(out=smT, in0=smT, in1=tmp2, op=ALU.add)
                smpl = offp.tile([128, 2, 128], bf16)
                tps = tp_ps.tile([128, 512], bf16, tag="tp4")
                for ci in range(2):
                    nc.tensor.transpose(tps[:, ts(ci, 128)], smT[:, ts(ci, 128)], ident16)
                nc.vector.tensor_copy(smpl.rearrange("p a b -> p (a b)"), tps[:, 0:256])
                v_t = offp.tile([128, 2, 128], bf16)
                for dst, wt, bt in ((k_t, pkw_t, pkb_t), (v_t, pvw_t, pvb_t)):
                    for mo in range(2):
                        ps = small_ps()
                        for ci in range(2):
                            nc.tensor.matmul(ps, wt[:, ci * 2 + mo, :], smpl[:, ci, :],
                                             start=(ci == 0), stop=(ci == 1))
                        nc.vector.tensor_scalar(out=dst[:, mo, :], in0=ps,
                                                scalar1=bt[:, mo:mo + 1], scalar2=None, op0=ALU.add)
                tpv = tp_ps.tile([128, 512], bf16, tag="tp4")
                for ci in range(2):
                    nc.tensor.transpose(tpv[:, ts(ci, 128)], v_t[:, ci, :], ident16)
                nc.vector.tensor_copy(vT8.rearrange("p a b -> p (a b)"), tpv[:, 0:256])

            # =======================================================
            # Phase B: attention + output projection
            # =======================================================
            with tc.tile_pool(name="apool", bufs=1) as apool, \
                 tc.tile_pool(name="epool", bufs=3) as epool, \
                 tc.tile_pool(name="npool", bufs=2) as npool, \
                 tc.tile_pool(name="qk_ps", bufs=3, space="PSUM") as qk_ps, \
                 tc.tile_pool(name="avs_ps", bufs=3, space="PSUM") as avs_ps, \
                 tc.tile_pool(name="po_ps", bufs=2, space="PSUM") as po_ps:

                att_t = apool.tile([128, 2, HW], bf16)
                E_tiles = {}

                def stage_qk(nb):
                    E = epool.tile([128, 8, 512], bf16, tag="E")
                    for hg4 in range(2):
                        for j in range(4):
                            qk = qk_ps.tile([128, 512], f32, tag="qk")
                            nc.tensor.matmul(qk, k_t[ts(j, 32), hg4, :],
                                             q_t[ts(j, 32), hg4, ts(nb, 512)],
                                             start=True, stop=True,
                                             tile_position=(32 * j, 0))
                            nc.scalar.activation(out=E[:, hg4 * 4 + j, :], in_=qk,
                                                 func=AF.Exp, scale=SCALE)
                    E_tiles[nb] = E

                def stage_av(nb):
                    E = E_tiles.pop(nb)
                    for g in range(2):
                        avg = avs_ps.tile([128, 512], f32, tag="avs")
                        for j in range(4):
                            nc.tensor.matmul(avg[ts(j, 32), :], vT8[:, g * 4 + j, :],
                                             E[:, g * 4 + j, :], start=True, stop=True,
                                             tile_position=(0, 32 * j))
                        sums = avs_ps.tile([128, 512], f32, tag="avs")
                        for j in range(4):
                            nc.tensor.matmul(sums[ts(j, 32), :], ones_m,
                                             E[:, g * 4 + j, :], start=True, stop=True,
                                             tile_position=(0, 32 * j))
                        rec = npool.tile([128, 512], f32, tag="rec")
                        nc.vector.reciprocal(out=rec, in_=sums)
                        nc.vector.tensor_tensor(out=att_t[:, g, ts(nb, 512)],
                                                in0=avg, in1=rec, op=ALU.mult)

                def stage_po(nb):
                    for mo in range(2):
                        ps = po_ps.tile([128, 512], f32, tag="po")
                        for ci in range(2):
                            nc.tensor.matmul(ps, pow_t[:, ci * 2 + mo, :],
                                             att_t[:, ci, ts(nb, 512)],
                                             start=(ci == 0), stop=(ci == 1))
                        ot = npool.tile([128, 512], f32, tag="ot")
                        nc.vector.tensor_scalar(out=ot, in0=ps, scalar1=pob_t[:, mo:mo + 1],
                                                scalar2=None, op0=ALU.add)
                        nc.sync.dma_start(out=out_d[ts(mo, 128), ts(nb, 512)], in_=ot)

                for step in range(10):
                    if step < 8:
                        stage_qk(step)
                    if 1 <= step <= 8:
                        stage_av(step - 1)
                    if step >= 2:
                        stage_po(step - 2)

    nc.finalize()
    return nc


def _host_prep(inp):
    import ml_dtypes
    e4 = ml_dtypes.float8_e4m3
    bf = ml_dtypes.bfloat16
    g = {k: np.ascontiguousarray(np.asarray(v, dtype=np.float32)) for k, v in inp.items()}
    s = g['bn_g'] / np.sqrt(g['bn_v'] + EPS)
    fwf = g['fuse_w'] * s[:, None, None, None]          # [256, 512, 3, 3]
    fbf = (g['fuse_b'] - g['bn_m']) * s + g['bn_b']
    M1 = g['sw1_w'] @ g['projq_w']
    c1 = g['sw1_w'] @ g['projq_b'] + g['sw1_b']

    def lhsT4(wmat, dt=bf):  # [out, in] -> [128, 4(ci*2+mo), 128]
        a = np.zeros((128, 4, 128), np.float32)
        for ci in range(2):
            for mo in range(2):
                a[:, ci * 2 + mo, :] = wmat[mo * 128:(mo + 1) * 128, ci * 128:(ci + 1) * 128].T
        return a.astype(dt)

    def b2(vec):  # [256] -> [128, 2]
        return np.stack([vec[0:128], vec[128:256]], 1).astype(np.float32)

    d = {}
    if CONV_FP8:
        fw8 = np.zeros((128, 9, 2, 2, 2, 128), np.float32)
        for ky in range(3):
            for kx in range(3):
                tap = ky * 3 + kx
                for img in range(2):
                    for mo in range(2):
                        for i in range(2):
                            ci = img * 2 + i
                            fw8[:, tap, img, mo, i, :] = \
                                fwf[mo * 128:(mo + 1) * 128, ci * 128:(ci + 1) * 128, ky, kx].T * WS
        d['fw8'] = fw8.astype(e4).view(np.uint8)
    else:
        fw16 = np.zeros((128, 9, 4, 2, 128), np.float32)
        for ky in range(3):
            for kx in range(3):
                tap = ky * 3 + kx
                for ci in range(4):
                    for mo in range(2):
                        fw16[:, tap, ci, mo, :] = \
                            fwf[mo * 128:(mo + 1) * 128, ci * 128:(ci + 1) * 128, ky, kx].T
        d['fw16'] = fw16.astype(bf)
    d['fb'] = b2(fbf)
    d['pqw'] = lhsT4(g['projq_w']); d['pqb'] = b2(g['projq_b'])
    d['m1w'] = lhsT4(M1);           d['c1b'] = b2(c1)
    sw2a = np.zeros((128, 2, 2), np.float32)
    for ci in range(2):
        sw2a[:, ci, :] = g['sw2_w'][:, ci * 128:(ci + 1) * 128].T
    d['sw2w'] = sw2a.astype(bf)
    db = float(g['sw2_b'][0] - g['sw2_b'][1])
    d['sigb'] = np.full((128, 1), db / 2.0, np.float32)
    for nm, key in (('pwx', 'offx_pw_w'), ('pwy', 'offy_pw_w')):
        a = np.zeros((128, 2, 2), np.float32)
        for ci in range(2):
            a[:, ci, :] = g[key][:, ci * 128:(ci + 1) * 128].T
        d[nm] = a
    dwsc = np.zeros((128, 2, 2, 81), np.float32)
    dwbc = np.zeros((128, 2, 2), np.float32)
    lnGc = np.zeros((128, 2, 2), np.float32)
    lnBc = np.zeros((128, 2, 2), np.float32)
    for img, pre in ((0, 'offx'), (1, 'offy')):
        w = g[pre + '_dw_w'][:, 0].reshape(256, 81)
        for ci in range(2):
            dwsc[:, ci, img, :] = w[ci * 128:(ci + 1) * 128]
            dwbc[:, img, ci] = g[pre + '_dw_b'][ci * 128:(ci + 1) * 128]
            lnGc[:, img, ci] = g[pre + '_ln_g'][ci * 128:(ci + 1) * 128]
            lnBc[:, img, ci] = g[pre + '_ln_b'][ci * 128:(ci + 1) * 128]
    d['dwsc'] = dwsc.astype(np.float16)
    d['dwbc'] = dwbc; d['lnGc'] = lnGc; d['lnBc'] = lnBc
    ry = (np.linspace(0.5, Hk - 0.5, Hk, dtype=np.float32) / np.float32(Hk - 1.0)) * 2.0 - 1.0
    gy, gx = np.meshgrid(ry, ry, indexing='ij')
    d['ref2'] = np.stack([gy, gx], 0).reshape(2, 64).astype(np.float32)
    d['pkw'] = lhsT4(g['projk_w']); d['pkb'] = b2(g['projk_b'])
    d['pvw'] = lhsT4(g['projv_w']); d['pvb'] = b2(g['projv_b'])
    d['pow'] = lhsT4(g['projo_w']); d['pob'] = b2(g['projo_b'])
    return g, d


def kernel(**inputs):
    import ml_dtypes
    from concourse.bass_utils import run_bass_kernel_spmd
    e4 = ml_dtypes.float8_e4m3
    bf = ml_dtypes.bfloat16

    if 'nc' not in _CACHE:
        _CACHE['nc'] = _build_program()
    nc = _CACHE['nc']

    g, wd = _host_prep(inputs)
    in_maps = []
    for b in range(B):
        m = dict(wd)
        xpad = np.zeros((128, 2, 2, PADR, PADR), np.float32)
        for img, key in ((0, 'x'), (1, 'y')):
            im = g[key][b]                                    # [256, 64, 64]
            for ci in range(2):
                xpad[:, img, ci, 4:68, 4:68] = im[ci * 128:(ci + 1) * 128]
            # bf16 gather table [5184, 256]
            t = np.zeros((PADR, PADR, C), np.float32)
            t[4:68, 4:68] = im.transpose(1, 2, 0)
            m['xT16' if img == 0 else 'yT16'] = t.reshape(NROW, C).astype(bf)
        xflat = xpad.reshape(128, 2, 2, NROW)
        if CONV_FP8:
            m['xp8'] = xflat.astype(e4).view(np.uint8)
        else:
            m['xp16'] = xflat.astype(bf)
        m['dwp16'] = xflat.astype(np.float16)
        in_maps.append(m)

    res = run_bass_kernel_spmd(nc, in_maps, list(range(B)))
    out = np.stack([res.results[i]['out'].reshape(C, H, W) for i in range(B)])
    return out.astype(np.float32)
